# revision 49
# baseline (speedup 1.0000x reference)
"""Trainium2 Bass kernel for the EnhancedEncoderLayer (dense MHA + low-rank
top-k sparse attention + FFN, two layernorms).

Sharding: 8 cores = (batch b in 0..3) x (query-half h in {0,1}). Each core
computes output rows [b, h*512:(h+1)*512, :]. K/V-side projections are
computed redundantly per batch pair (no cross-core communication).

The host permutes src[b].T columns so each core's own query tokens are
columns 0..511 (attention contracts over all keys, so key order is
irrelevant); this keeps the SPMD program identical across cores.

v2 design notes:
- v/vsp projections are x-stationary (lhsT = xT chunk), producing
  token-major Vaug/Vsp directly -- no PE transposes, no ACT copies.
- dense attention is software-pipelined: scores(h) / proj filler /
  ctx(h-1), so ACT exp latency never stalls the in-order PE queue.
- the sparse path runs in bf16: exp writes bf16 psp, the top-k threshold
  bisection scans at 2x DVE rate with 18 iterations, spmm is bf16.
- outproj+spmm+fuse+LN1 run qt-outer so LN1 overlaps matmuls; ff2 is
  qt-outer against an SBUF-resident f2T with LN2+output pipelined per qt.
- ln1 gamma/beta are folded into ff1 weights/bias host-side.
- all host tensors are laid out so every DMA is contiguous per partition.
"""
import sys
import os
import contextlib

for _p in ('/opt/trn_rl_repo',):
    if _p not in sys.path:
        sys.path.insert(0, _p)

import numpy as np
import concourse.bacc as bacc
import concourse.tile as tile
from concourse import mybir
from concourse.bass_utils import run_bass_kernel_spmd
from concourse.masks import make_identity

F32 = mybir.dt.float32
F32R = mybir.dt.float32r
BF16 = mybir.dt.bfloat16
AF = mybir.ActivationFunctionType
OP = mybir.AluOpType

B, S, D, H, R, DFF = 4, 1024, 1024, 16, 64, 4096
DH = D // H          # 64
SQ = S // 2          # 512 own queries per core
KK = max(1, int(S * 0.2))   # 204
KC = D // 128        # 8 contraction chunks over D
FC = DFF // 128      # 32 chunks over DFF
NQT = SQ // 128      # 4 query tiles
NTOK = S // 128      # 8 token tiles
BISECT_ITERS = 19
INV_SQRT = 0.125     # 1/sqrt(DH) == 1/sqrt(R)

_cached = {}


def _build():
    nc = bacc.Bacc()

    def din(name, shape, dt=F32):
        return nc.declare_dram_parameter(name, list(shape), dt, isOutput=False)

    xT = din("xT", [D, S], BF16)      # src[b].T, own-query columns first
    x_own = din("x_own", [SQ, D])     # own rows, token-major
    wqkvT = din("wqkvT", [D, 3 * D], BF16)
    woT = din("woT", [D, D], BF16)
    vpT = din("vpT", [D, D], BF16)
    qkp_pm = din("qkp_pm", [128, KC * 2 * R], BF16)   # partition-major packed
    f1T = din("f1T", [D, DFF], BF16)  # pre-scaled by ln1 gamma
    f2T = din("f2T", [DFF, D], BF16)
    # packed per-partition bias columns: [0:8]=q, [8:16]=k, [16:48]=b1_eff,
    # [48]=bqp (rows 0:64), [49]=bkp (rows 0:64)
    bcols = din("bcols", [128, 50])
    # packed broadcast rows: [bv, bvp, bo, b12, g1, g2, be2]
    brow = din("brow", [1, 7 * D])
    lam = din("lam", [1, 1])
    out = nc.declare_dram_parameter("out", [SQ, D], F32, isOutput=True)

    xT_r = xT.ap().rearrange("(kc p) s -> p kc s", p=128)
    wqkvT_r = wqkvT.ap().rearrange("(kc p) f -> p kc f", p=128)
    woT_r = woT.ap().rearrange("(kc p) f -> p kc f", p=128)
    vpT_r = vpT.ap().rearrange("(kc p) f -> p kc f", p=128)
    qkp_r = qkp_pm.ap().rearrange("p (kc f) -> p kc f", f=2 * R)
    f1T_r = f1T.ap().rearrange("(kc p) f -> p kc f", p=128)
    f2T_r = f2T.ap().rearrange("(kc p) f -> p kc f", p=128)

    with tile.TileContext(nc) as tc:
        est = contextlib.ExitStack()
        with est:
            # ---------------- constants ----------------
            consts = est.enter_context(tc.tile_pool(name="consts", bufs=1))

            ident_f = consts.tile([128, 128], F32, name="ident_f")
            make_identity(nc, ident_f)
            ident_b = consts.tile([128, 128], BF16, name="ident_b")
            nc.vector.tensor_copy(out=ident_b, in_=ident_f)

            eps_t = consts.tile([128, 1], F32, name="eps_t")
            nc.vector.memset(eps_t, 1e-5)
            ones1 = consts.tile([128, 1], F32, name="ones1")
            nc.vector.memset(ones1, 1.0)
            ones1b = consts.tile([128, 1], BF16, name="ones1b")
            nc.vector.memset(ones1b, 1.0)
            ones16b = consts.tile([128, 16], BF16, name="ones16b")
            nc.vector.memset(ones16b, 1.0)

            lam_t = consts.tile([1, 1], F32, name="lam_t")
            nc.sync.dma_start(out=lam_t, in_=lam.ap())
            sg_t = consts.tile([1, 1], F32, name="sg_t")
            nc.scalar.activation(out=sg_t, in_=lam_t, func=AF.Sigmoid)
            sig_bc = consts.tile([128, 1], F32, name="sig_bc")
            nc.gpsimd.partition_broadcast(sig_bc, sg_t)
            oms_bc = consts.tile([128, 1], F32, name="oms_bc")
            nc.vector.tensor_sub(oms_bc, ones1, sig_bc)

            # packed bias columns (one contiguous DMA)
            bcols_t = consts.tile([128, 50], F32, name="bcols_t")
            nc.gpsimd.dma_start(out=bcols_t, in_=bcols.ap())
            bq_c = bcols_t[:, 0:8]
            bk_c = bcols_t[:, 8:16]
            b1_c = bcols_t[:, 16:48]
            bqp_c = bcols_t[0:64, 48:49]
            bkp_c = bcols_t[0:64, 49:50]

            # bisect state
            bis = est.enter_context(tc.tile_pool(name="bis", bufs=1))
            lo = bis.tile([128, NQT], F32, name="lo")
            hi = bis.tile([128, NQT], F32, name="hi")
            mid = bis.tile([128, NQT], F32, name="mid")
            cnts = bis.tile([128, NQT], F32, name="cnts")
            pred = bis.tile([128, NQT], mybir.dt.uint32, name="pred")
            rs_sp = bis.tile([128, NQT], F32, name="rs_sp")
            rcp_sp = bis.tile([128, NQT], F32, name="rcp_sp")

            # long-lived activations
            sp_stack = contextlib.ExitStack()
            sp_pool = sp_stack.enter_context(
                tc.tile_pool(name="sp_pool", bufs=1))
            psp = sp_pool.tile([128, NQT, S], BF16, name="psp")
            kspT = sp_pool.tile([64, S], F32R, name="kspT")
            qspT = sp_pool.tile([64, SQ], F32R, name="qspT")

            av_stack = contextlib.ExitStack()
            av_pool = av_stack.enter_context(
                tc.tile_pool(name="av_pool", bufs=1))
            Vaug = av_pool.tile([128, NTOK, H * (DH + 1)], BF16, name="Vaug")
            Vsp = av_pool.tile([128, NTOK, D], BF16, name="Vsp")
            ctxT = av_pool.tile([128, KC, SQ], BF16, name="ctxT")

            Vaug_h = Vaug.rearrange("p t (h c) -> p t h c", c=DH + 1)
            for t in range(NTOK):
                nc.vector.tensor_copy(out=Vaug_h[:, t, :, DH:DH + 1],
                                      in_=ones16b)

            xot_stack = contextlib.ExitStack()
            xot_pool = xot_stack.enter_context(tc.tile_pool(name="xot_pool",
                                                            bufs=1))
            xot = xot_pool.tile([128, NQT, D], F32, name="xot")

            # out_proj weights (DMA issued later, after the startup crunch)
            wo_stack = contextlib.ExitStack()
            wo_pool = wo_stack.enter_context(
                tc.tile_pool(name="wo_pool", bufs=1))
            woT_s = wo_pool.tile([128, KC, D], BF16, name="woT_s")

            # =========== phase 0/1: input loads, sparse + v projections ====
            xbf_stack = contextlib.ExitStack()
            xbf_pool = xbf_stack.enter_context(
                tc.tile_pool(name="xbf_pool", bufs=1))
            xbf = xbf_pool.tile([128, KC, S], BF16, name="xbf")

            # early broadcast rows: bv, bvp, sig*bo
            early_stack = contextlib.ExitStack()
            early_bc = early_stack.enter_context(
                tc.tile_pool(name="early_bc", bufs=1))
            brow_t = early_bc.tile([1, 3 * D], F32, name="brow_t")
            nc.gpsimd.dma_start(out=brow_t, in_=brow.ap()[:, 0:3 * D])
            bv_bc = early_bc.tile([128, D], F32, name="bv_bc")
            nc.gpsimd.partition_broadcast(bv_bc, brow_t[:, 0:D])
            bvp_bc = early_bc.tile([128, D], F32, name="bvp_bc")
            nc.gpsimd.partition_broadcast(bvp_bc, brow_t[:, D:2 * D])
            bo_sig = early_bc.tile([128, D], F32, name="bo_sig")
            nc.gpsimd.partition_broadcast(bo_sig, brow_t[:, 2 * D:3 * D])
            nc.vector.tensor_scalar_mul(bo_sig, bo_sig, sig_bc)

            with contextlib.ExitStack() as ph0:
                wsp_pool = ph0.enter_context(
                    tc.tile_pool(name="wsp_pool", bufs=1))
                ps_proj = ph0.enter_context(
                    tc.tile_pool(name="ps_proj", bufs=3, space="PSUM"))

                qkpt = wsp_pool.tile([128, KC, 2 * R], BF16, name="qkpt")
                nc.sync.dma_start(out=qkpt, in_=qkp_r)
                qpt = qkpt[:, :, 0:R]
                kpt = qkpt[:, :, R:2 * R]
                for kc in range(KC):
                    eng = nc.scalar if kc % 2 == 0 else nc.sync
                    eng.dma_start(out=xbf[:, kc, :], in_=xT_r[:, kc, :])

                # ---- sparse projections + scores ----
                with nc.named_scope("p0_ksp_qsp"):
                    for nh in range(2):
                        ps = ps_proj.tile([128, 512], F32, name="ps",
                                          tag="ps")
                        for kc in range(KC):
                            nc.tensor.matmul(
                                ps[0:64, :], kpt[:, kc, :],
                                xbf[:, kc, nh * 512:nh * 512 + 512],
                                start=(kc == 0), stop=(kc == KC - 1))
                        nc.scalar.activation(
                            out=kspT[:, nh * 512:nh * 512 + 512],
                            in_=ps[0:64, :], func=AF.Identity, bias=bkp_c,
                            scale=1.0)
                    ps = ps_proj.tile([128, 512], F32, name="ps", tag="ps")
                    for kc in range(KC):
                        nc.tensor.matmul(ps[0:64, :], qpt[:, kc, :],
                                         xbf[:, kc, 0:SQ],
                                         start=(kc == 0), stop=(kc == KC - 1))
                    nc.scalar.activation(out=qspT, in_=ps[0:64, :],
                                         func=AF.Identity, bias=bqp_c,
                                         scale=1.0)

                with nc.named_scope("p2_ssp"):
                    for qt in range(NQT):
                        for nh in range(2):
                            ps = ps_proj.tile([128, 512], F32, name="ps",
                                              tag="ps")
                            nc.tensor.matmul(
                                ps, qspT[:, qt * 128:qt * 128 + 128],
                                kspT[:, nh * 512:nh * 512 + 512],
                                start=True, stop=True)
                            nc.scalar.activation(
                                out=psp[:, qt, nh * 512:nh * 512 + 512],
                                in_=ps, func=AF.Exp, scale=INV_SQRT)

                # own-token residual (+ sig*bo)
                for qt in range(NQT):
                    nc.scalar.dma_start(
                        out=xot[:, qt, :],
                        in_=x_own.ap()[qt * 128:qt * 128 + 128, :])
                    nc.gpsimd.tensor_add(xot[:, qt, :], xot[:, qt, :],
                                         bo_sig)

            # ---- v/vsp x-stationary projections -> token-major ----
            with contextlib.ExitStack() as ph4:
                # reopened weight pool (wv_s, wvp_s still live via av? no --
                # keep them in this scope)
                wv_pool2 = ph4.enter_context(
                    tc.tile_pool(name="wv_pool2", bufs=1))
                wv_s = wv_pool2.tile([128, KC, D], BF16, name="wv_s2")
                wvp_s = wv_pool2.tile([128, KC, D], BF16, name="wvp_s2")
                for kc in range(KC):
                    nc.sync.dma_start(out=wv_s[:, kc, :],
                                      in_=wqkvT_r[:, kc, 2 * D:3 * D])
                    nc.sync.dma_start(out=wvp_s[:, kc, :],
                                      in_=vpT_r[:, kc, :])
                ps_v = ph4.enter_context(
                    tc.tile_pool(name="ps_v", bufs=8, space="PSUM"))
                with nc.named_scope("p4_v"):
                    for t in range(NTOK):
                        pva0 = ps_v.tile([128, 512], F32, name="pv", tag="pv")
                        pva1 = ps_v.tile([128, 512], F32, name="pv", tag="pv")
                        pvs0 = ps_v.tile([128, 512], F32, name="pv", tag="pv")
                        pvs1 = ps_v.tile([128, 512], F32, name="pv", tag="pv")
                        for kc in range(KC):
                            xck = xbf[:, kc, t * 128:t * 128 + 128]
                            st = (kc == 0)
                            sp = (kc == KC - 1)
                            nc.tensor.matmul(pva0, xck, wv_s[:, kc, 0:512],
                                             start=st, stop=sp)
                            nc.tensor.matmul(pva1, xck, wv_s[:, kc, 512:1024],
                                             start=st, stop=sp)
                            nc.tensor.matmul(pvs0, xck, wvp_s[:, kc, 0:512],
                                             start=st, stop=sp)
                            nc.tensor.matmul(pvs1, xck, wvp_s[:, kc, 512:1024],
                                             start=st, stop=sp)
                        nc.vector.tensor_add(
                            Vaug_h[:, t, 0:8, 0:DH], pva0, bv_bc[:, 0:512])
                        nc.vector.tensor_add(
                            Vaug_h[:, t, 8:16, 0:DH], pva1, bv_bc[:, 512:1024])
                        nc.vector.tensor_add(
                            Vsp[:, t, 0:512], pvs0, bvp_bc[:, 0:512])
                        nc.vector.tensor_add(
                            Vsp[:, t, 512:1024], pvs1, bvp_bc[:, 512:1024])
            early_stack.close()   # free bv_bc, bvp_bc, bo_sig

            # out_proj weights resident (used in p6)
            nc.sync.dma_start(out=woT_s, in_=woT_r)

            # bisect scratch: lives until after the masking pass
            scr_stack = contextlib.ExitStack()
            scr_pool = scr_stack.enter_context(
                tc.tile_pool(name="scr", bufs=4))

            def bisect_iter():
                # one threshold-bisection step; qt 0-2 scan on DVE, qt 3 on
                # GpSimd (SBUF-only engine, otherwise idle here)
                nc.vector.tensor_add(mid, lo, hi)
                nc.vector.tensor_scalar_mul(mid, mid, 0.5)
                for qt in range(NQT):
                    scr = scr_pool.tile([128, S], BF16, name="scr",
                                        tag="scr")
                    nc.vector.scalar_tensor_tensor(
                        out=scr, in0=psp[:, qt, :],
                        scalar=mid[:, qt:qt + 1],
                        in1=ones1b.to_broadcast([128, S]),
                        op0=OP.is_ge, op1=OP.mult,
                        accum_out=cnts[:, qt:qt + 1])
                nc.vector.tensor_scalar(out=pred, in0=cnts,
                                        scalar1=float(KK),
                                        scalar2=None, op0=OP.is_ge)
                nc.vector.copy_predicated(lo, pred, mid)
                nc.vector.tensor_scalar(out=pred, in0=cnts,
                                        scalar1=float(KK),
                                        scalar2=None, op0=OP.is_lt)
                nc.vector.copy_predicated(hi, pred, mid)

            # ======== phase 5: k/q projections + pipelined attention =======
            kq_stack = contextlib.ExitStack()
            kq_pool = kq_stack.enter_context(
                tc.tile_pool(name="kq_pool", bufs=1))
            kT = kq_pool.tile([128, KC, S], BF16, name="kT")
            qT = kq_pool.tile([128, KC, SQ], BF16, name="qT")
            with contextlib.ExitStack() as ph5:
                wstr = ph5.enter_context(tc.tile_pool(name="wstr", bufs=3))
                pt_pool = ph5.enter_context(
                    tc.tile_pool(name="pt_pool", bufs=16))
                rc_pool = ph5.enter_context(
                    tc.tile_pool(name="rc_pool", bufs=2))
                ps_kq = ph5.enter_context(
                    tc.tile_pool(name="ps_kq", bufs=2, space="PSUM"))
                ps_attn = ph5.enter_context(
                    tc.tile_pool(name="ps_attn", bufs=4, space="PSUM"))
                ps_ctx = ph5.enter_context(
                    tc.tile_pool(name="ps_ctx", bufs=2, space="PSUM"))

                pts = {}

                def proj_piece(ft):
                    wk = wstr.tile([128, KC, 128], BF16, name="wk", tag="wk")
                    nc.sync.dma_start(
                        out=wk, in_=wqkvT_r[:, :, D + ft * 128:D + ft * 128 + 128])
                    wq = wstr.tile([128, KC, 128], BF16, name="wq", tag="wq")
                    nc.sync.dma_start(
                        out=wq, in_=wqkvT_r[:, :, ft * 128:ft * 128 + 128])
                    for nh in range(2):
                        ps = ps_kq.tile([128, 512], F32, name="pkq",
                                        tag="pkq")
                        for kc in range(KC):
                            nc.tensor.matmul(
                                ps, wk[:, kc, :],
                                xbf[:, kc, nh * 512:nh * 512 + 512],
                                start=(kc == 0), stop=(kc == KC - 1))
                        nc.scalar.activation(
                            out=kT[:, ft, nh * 512:nh * 512 + 512],
                            in_=ps, func=AF.Identity,
                            bias=bk_c[:, ft:ft + 1], scale=1.0)
                    ps = ps_kq.tile([128, 512], F32, name="pkq", tag="pkq")
                    for kc in range(KC):
                        nc.tensor.matmul(ps, wq[:, kc, :], xbf[:, kc, 0:SQ],
                                         start=(kc == 0), stop=(kc == KC - 1))
                    nc.scalar.activation(
                        out=qT[:, ft, :], in_=ps, func=AF.Identity,
                        bias=bq_c[:, ft:ft + 1], scale=1.0)

                def scores_pair(h0):
                    # heads h0 (rows 0:64) and h0+1 (rows 64:128) issue
                    # interleaved -- disjoint PE row groups run concurrently
                    ft = h0 // 2
                    pts[h0] = []
                    pts[h0 + 1] = []
                    for t in range(NTOK):
                        for hh in (h0, h0 + 1):
                            po = 64 * (hh % 2)
                            ps = ps_attn.tile([128, 512], F32, name="ps_s",
                                              tag="ps_s")
                            nc.tensor.matmul(
                                ps, kT[po:po + 64, ft, t * 128:t * 128 + 128],
                                qT[po:po + 64, ft, :], start=True, stop=True)
                            pt = pt_pool.tile([128, 512], BF16, name="pT",
                                              tag="pT")
                            nc.scalar.activation(out=pt, in_=ps, func=AF.Exp,
                                                 scale=INV_SQRT)
                            pts[hh].append(pt)

                def ctx(hh):
                    po = 64 * (hh % 2)
                    ft = hh // 2
                    pctx = ps_ctx.tile([128, 512], F32, name="ps_c",
                                       tag="ps_c")
                    for t in range(NTOK):
                        nc.tensor.matmul(
                            pctx[0:65, :], Vaug_h[:, t, hh, 0:DH + 1],
                            pts[hh][t], start=(t == 0), stop=(t == NTOK - 1))
                    rsr = rc_pool.tile([1, 512], F32, name="rsr", tag="rsr")
                    nc.vector.tensor_copy(out=rsr, in_=pctx[64:65, :])
                    rch = rc_pool.tile([1, 512], F32, name="rch", tag="rch")
                    nc.vector.reciprocal_approx_fast(out=rch, in_=rsr)
                    rb = rc_pool.tile([64, 512], F32, name="rb", tag="rb")
                    nc.gpsimd.partition_broadcast(rb, rch)
                    nc.vector.tensor_mul(out=ctxT[po:po + 64, ft, :],
                                         in0=pctx[0:64, :], in1=rb)
                    del pts[hh]

                with nc.named_scope("p5_kq_attn"):
                    nc.vector.memset(lo, 0.0)
                    nc.vector.memset(hi, 16.0)
                    proj_piece(0)
                    proj_piece(1)
                    bisect_iter()
                    for p in range(H // 2):
                        if p + 2 < KC:
                            proj_piece(p + 2)
                        scores_pair(2 * p)
                        if p >= 1:
                            ctx(2 * p - 2)
                            ctx(2 * p - 1)
                        bisect_iter()
                        bisect_iter()
                    ctx(H - 2)
                    ctx(H - 1)

                # final masking + renorm scale for the sparse path
                with nc.named_scope("p3_mask"):
                    for qt in range(NQT):
                        nc.vector.scalar_tensor_tensor(
                            out=psp[:, qt, :], in0=psp[:, qt, :],
                            scalar=lo[:, qt:qt + 1],
                            in1=psp[:, qt, :], op0=OP.is_ge, op1=OP.mult,
                            accum_out=rs_sp[:, qt:qt + 1])
                    nc.vector.tensor_scalar(out=rs_sp, in0=rs_sp,
                                            scalar1=1e-9, scalar2=None,
                                            op0=OP.add)
                    nc.vector.reciprocal(rcp_sp, rs_sp)
                    nc.vector.tensor_scalar_mul(rcp_sp, rcp_sp, oms_bc)

            kq_stack.close()    # free kT, qT
            scr_stack.close()
            xbf_stack.close()   # free xbf

            # ========= phase 6: outproj + spmm + fuse + LN1 (qt-outer) =====
            # late broadcast rows: b12, g1, g2, be2 (right-side stack)
            late_bc = est.enter_context(
                tc.tile_pool(name="late_bc", bufs=1, side="right"))
            brow_t2 = late_bc.tile([1, 4 * D], F32, name="brow_t2")
            nc.gpsimd.dma_start(out=brow_t2, in_=brow.ap()[:, 3 * D:7 * D])
            b12_bc = late_bc.tile([128, D], F32, name="b12_bc")
            g1_bc = late_bc.tile([128, D], F32, name="g1_bc")
            g2_bc = late_bc.tile([128, D], F32, name="g2_bc")
            be2_bc = late_bc.tile([128, D], F32, name="be2_bc")
            for i, t_bc in enumerate([b12_bc, g1_bc, g2_bc, be2_bc]):
                nc.gpsimd.partition_broadcast(
                    t_bc, brow_t2[:, i * D:(i + 1) * D])

            fse = est.enter_context(tc.tile_pool(name="fse", bufs=1,
                                                 side="right"))
            x1 = fse.tile([128, NQT, D], F32, name="x1")
            mv2 = fse.tile([128, NQT, 2], F32, name="mv2")
            stats = fse.tile([128, NQT, 2, 6], F32, name="stats")
            sd = fse.tile([128, NQT], F32, name="sd")
            rstd = fse.tile([128, NQT], F32, name="rstd")

            xln_stack = contextlib.ExitStack()
            xlnT_pool = xln_stack.enter_context(
                tc.tile_pool(name="xlnT_pool", bufs=1, side="right"))
            xlnT = xlnT_pool.tile([128, KC, SQ], BF16, name="xlnT")
            w3_stack = contextlib.ExitStack()
            w3str = w3_stack.enter_context(
                tc.tile_pool(name="w3str", bufs=4, side="right"))

            def w1_chunk(jj):
                wt = w3str.tile([128, KC, 256], BF16, name="w1t", tag="w3")
                eng = nc.scalar if jj % 2 == 0 else nc.sync
                eng.dma_start(out=wt, in_=f1T_r[:, :, jj * 256:jj * 256 + 256])
                return wt

            w1_tiles = {jj: w1_chunk(jj) for jj in range(4)}

            xbf1_stack = contextlib.ExitStack()
            xbf1_pool = xbf1_stack.enter_context(
                tc.tile_pool(name="xbf1_pool", bufs=1, side="right"))
            xbf1 = xbf1_pool.tile([128, NQT, D], BF16, name="xbf1")

            def ln_stats(src_ap, qt):
                for half in range(2):
                    nc.vector.bn_stats(
                        out=stats[:, qt, half, :],
                        in_=src_ap[:, half * 512:half * 512 + 512])
                nc.vector.bn_aggr(out=mv2[:, qt, :], in_=stats[:, qt])
                nc.scalar.activation(out=sd[:, qt:qt + 1],
                                     in_=mv2[:, qt, 1:2], func=AF.Sqrt,
                                     bias=eps_t, scale=1.0)
                nc.vector.reciprocal(rstd[:, qt:qt + 1], sd[:, qt:qt + 1])

            with contextlib.ExitStack() as ph6:
                pm_pool = ph6.enter_context(tc.tile_pool(name="pm_pool",
                                                         bufs=2))
                ps_o = ph6.enter_context(
                    tc.tile_pool(name="ps_o", bufs=3, space="PSUM"))
                ps_sp = ph6.enter_context(
                    tc.tile_pool(name="ps_sp", bufs=3, space="PSUM"))
                ps_tr = ph6.enter_context(
                    tc.tile_pool(name="ps_tr", bufs=2, space="PSUM"))
                def xln_transpose(qt):
                    # transpose normalized qt block for ff1 (lagged one qt
                    # so the PE never waits on LN1's DVE chain)
                    qc = slice(qt * 128, qt * 128 + 128)
                    for fc in range(KC):
                        pst = ps_tr.tile([128, 128], BF16, name="pst",
                                         tag="pst")
                        nc.tensor.transpose(
                            pst, xbf1[:, qt, fc * 128:fc * 128 + 128],
                            ident_b)
                        nc.vector.tensor_copy(out=xlnT[:, fc, qc],
                                              in_=pst)

                with nc.named_scope("p6_fuse"):
                    for qt in range(NQT):
                        qc = slice(qt * 128, qt * 128 + 128)
                        # out_proj (2 halves, ctxT-stationary)
                        po0 = ps_o.tile([128, 512], F32, name="po", tag="po")
                        po1 = ps_o.tile([128, 512], F32, name="po", tag="po")
                        for kc in range(KC):
                            st, sp = (kc == 0), (kc == KC - 1)
                            nc.tensor.matmul(po0, ctxT[:, kc, qc],
                                             woT_s[:, kc, 0:512],
                                             start=st, stop=sp)
                            nc.tensor.matmul(po1, ctxT[:, kc, qc],
                                             woT_s[:, kc, 512:1024],
                                             start=st, stop=sp)
                        # masked-p transposes for this qt
                        pmt = pm_pool.tile([128, NTOK, 128], BF16, name="pmt",
                                           tag="pmt")
                        for t in range(NTOK):
                            pst = ps_tr.tile([128, 128], BF16, name="pst",
                                             tag="pst")
                            nc.tensor.transpose(
                                pst, psp[:, qt, t * 128:t * 128 + 128],
                                ident_b)
                            nc.vector.tensor_copy(out=pmt[:, t, :], in_=pst)
                        # spmm (2 halves)
                        sp0 = ps_sp.tile([128, 512], F32, name="psp2",
                                         tag="psp2")
                        sp1 = ps_sp.tile([128, 512], F32, name="psp2",
                                         tag="psp2")
                        for t in range(NTOK):
                            st, spl = (t == 0), (t == NTOK - 1)
                            nc.tensor.matmul(sp0, pmt[:, t, :],
                                             Vsp[:, t, 0:512],
                                             start=st, stop=spl)
                            nc.tensor.matmul(sp1, pmt[:, t, :],
                                             Vsp[:, t, 512:1024],
                                             start=st, stop=spl)
                        if qt >= 1:
                            xln_transpose(qt - 1)
                        # fuse on DVE: x1 = sig*dense + rcp*spmm + xot
                        xq = x1[:, qt, :]
                        nc.vector.tensor_scalar(
                            out=xq[:, 0:512], in0=po0, scalar1=sig_bc,
                            scalar2=None, op0=OP.mult)
                        nc.vector.tensor_scalar(
                            out=xq[:, 512:1024], in0=po1, scalar1=sig_bc,
                            scalar2=None, op0=OP.mult)
                        nc.vector.tensor_add(xq, xq, xot[:, qt, :])
                        nc.vector.scalar_tensor_tensor(
                            out=xq[:, 0:512], in0=sp0,
                            scalar=rcp_sp[:, qt:qt + 1],
                            in1=xq[:, 0:512], op0=OP.mult, op1=OP.add)
                        nc.vector.scalar_tensor_tensor(
                            out=xq[:, 512:1024], in0=sp1,
                            scalar=rcp_sp[:, qt:qt + 1],
                            in1=xq[:, 512:1024], op0=OP.mult, op1=OP.add)
                        # LN1 (keep x1 raw f32 for the ff2 residual)
                        ln_stats(xq, qt)
                        nc.vector.tensor_scalar(
                            out=xbf1[:, qt, :], in0=xq,
                            scalar1=mv2[:, qt, 0:1],
                            scalar2=rstd[:, qt:qt + 1],
                            op0=OP.subtract, op1=OP.mult)
                    xln_transpose(NQT - 1)

            xbf1_stack.close()
            wo_stack.close()
            xot_stack.close()
            av_stack.close()   # free Vaug, Vsp, ctxT
            sp_stack.close()   # free psp, kspT, qspT

            # f2T resident for qt-outer ff2 (chunk DMAs spread through ff1)
            f2_stack = contextlib.ExitStack()
            f2_pool = f2_stack.enter_context(
                tc.tile_pool(name="f2_pool", bufs=1))
            f2_s = f2_pool.tile([128, FC, D], BF16, name="f2_s")

            # xg = xhat*g1 + (be1+b2), computed on DVE during ff1
            xg = fse.tile([128, NQT, D], F32, name="xg")

            # ============ ff1 + relu ============
            h1_stack = contextlib.ExitStack()
            h1_pool = h1_stack.enter_context(
                tc.tile_pool(name="h1_pool", bufs=1))
            h1T = h1_pool.tile([128, FC, SQ], BF16, name="h1T")
            with contextlib.ExitStack() as ph9:
                ps_f1 = ph9.enter_context(
                    tc.tile_pool(name="ps_f1", bufs=4, space="PSUM"))
                with nc.named_scope("p9_ff1"):
                    for jj in range(16):
                        wt = w1_tiles.pop(jj)
                        if jj + 4 < 16:
                            w1_tiles[jj + 4] = w1_chunk(jj + 4)
                        for kc2 in range(2):
                            nc.gpsimd.dma_start(
                                out=f2_s[:, jj * 2 + kc2, :],
                                in_=f2T_r[:, jj * 2 + kc2, :])
                        for fi in range(2):
                            dft = jj * 2 + fi
                            ps = ps_f1.tile([128, 512], F32, name="ps_f",
                                            tag="ps_f")
                            for kc in range(KC):
                                nc.tensor.matmul(
                                    ps, wt[:, kc, fi * 128:fi * 128 + 128],
                                    xlnT[:, kc, :],
                                    start=(kc == 0), stop=(kc == KC - 1))
                            nc.scalar.activation(
                                out=h1T[:, dft, :], in_=ps, func=AF.Relu,
                                bias=b1_c[:, dft:dft + 1], scale=1.0)
                        if jj < 2 * NQT and jj % 2 == 1:
                            # xg for qt = jj//2, hidden under ff1
                            qt = jj // 2
                            nc.vector.tensor_scalar(
                                out=xg[:, qt, :], in0=x1[:, qt, :],
                                scalar1=mv2[:, qt, 0:1],
                                scalar2=rstd[:, qt:qt + 1],
                                op0=OP.subtract, op1=OP.mult)
                            nc.vector.tensor_mul(xg[:, qt, :], xg[:, qt, :],
                                                 g1_bc)
                            nc.vector.tensor_add(xg[:, qt, :], xg[:, qt, :],
                                                 b12_bc)
            w3_stack.close()
            xln_stack.close()

            # ============ ff2 (qt-outer) + residual + LN2 + out ============
            with contextlib.ExitStack() as ph10:
                ps_f2 = ph10.enter_context(
                    tc.tile_pool(name="ps_f2", bufs=4, space="PSUM"))
                ot_pool = ph10.enter_context(
                    tc.tile_pool(name="ot_pool", bufs=2))
                with nc.named_scope("p10_ff2"):
                    for qt in range(NQT):
                        qc = slice(qt * 128, qt * 128 + 128)
                        pg0 = ps_f2.tile([128, 512], F32, name="pg", tag="pg")
                        pg1 = ps_f2.tile([128, 512], F32, name="pg", tag="pg")
                        for kc in range(FC):
                            st, sp = (kc == 0), (kc == FC - 1)
                            nc.tensor.matmul(pg0, h1T[:, kc, qc],
                                             f2_s[:, kc, 0:512],
                                             start=st, stop=sp)
                            nc.tensor.matmul(pg1, h1T[:, kc, qc],
                                             f2_s[:, kc, 512:1024],
                                             start=st, stop=sp)
                        x2 = x1[:, qt, :]
                        nc.vector.tensor_add(x2[:, 0:512], pg0,
                                             xg[:, qt, 0:512])
                        nc.vector.tensor_add(x2[:, 512:1024], pg1,
                                             xg[:, qt, 512:1024])
                        ln_stats(x2, qt)
                        ot = ot_pool.tile([128, D], F32, name="out_t",
                                          tag="out_t")
                        nc.vector.tensor_scalar(
                            out=ot, in0=x2, scalar1=mv2[:, qt, 0:1],
                            scalar2=rstd[:, qt:qt + 1],
                            op0=OP.subtract, op1=OP.mult)
                        nc.vector.tensor_mul(ot, ot, g2_bc)
                        nc.vector.tensor_add(ot, ot, be2_bc)
                        nc.scalar.dma_start(
                            out=out.ap()[qt * 128:qt * 128 + 128, :], in_=ot)
            h1_stack.close()
            f2_stack.close()

    nc.compile()
    return nc


def _prep_inputs(src, in_proj_w, in_proj_b, out_proj_w, out_proj_b,
                 Qp_w, Qp_b, Kp_w, Kp_b, Vp_w, Vp_b, lam,
                 ff1_w, ff1_b, ff2_w, ff2_b, ln1_g, ln1_b, ln2_g, ln2_b):
    import ml_dtypes
    f = np.float32
    A = lambda x: np.ascontiguousarray(x, dtype=f)
    AB = lambda x: np.ascontiguousarray(np.asarray(x, dtype=f),
                                        dtype=ml_dtypes.bfloat16)
    in_proj_w = np.asarray(in_proj_w, dtype=f)
    ff1_w = np.asarray(ff1_w, dtype=f)
    ln1_g = np.asarray(ln1_g, dtype=f)
    ln1_b = np.asarray(ln1_b, dtype=f)
    # fold ln1 gamma into ff1 weights, ln1 beta into ff1 bias
    f1_eff = ff1_w * ln1_g[None, :]            # [DFF, D]
    b1_eff = np.asarray(ff1_b, dtype=f) + ff1_w @ ln1_b

    # packed per-partition bias columns [128, 50]
    bcols = np.zeros((128, 50), dtype=f)
    qk_bias = np.asarray(in_proj_b, dtype=f)
    bcols[:, 0:8] = qk_bias[0:D].reshape(8, 128).T
    bcols[:, 8:16] = qk_bias[D:2 * D].reshape(8, 128).T
    bcols[:, 16:48] = b1_eff.reshape(32, 128).T
    bcols[0:64, 48] = np.asarray(Qp_b, dtype=f)
    bcols[0:64, 49] = np.asarray(Kp_b, dtype=f)

    # packed broadcast rows [1, 7D]: bv, bvp, bo, b12, g1, g2, be2
    brow = np.concatenate([
        qk_bias[2 * D:3 * D],
        np.asarray(Vp_b, dtype=f),
        np.asarray(out_proj_b, dtype=f),
        ln1_b + np.asarray(ff2_b, dtype=f),
        ln1_g,
        np.asarray(ln2_g, dtype=f),
        np.asarray(ln2_b, dtype=f),
    ]).reshape(1, 7 * D)

    # qkp packed partition-major: [128, kc, 2R] contiguous
    qkpT = np.concatenate([np.asarray(Qp_w).T, np.asarray(Kp_w).T], axis=1)
    qkp_pm = np.ascontiguousarray(
        np.asarray(qkpT, dtype=f).reshape(KC, 128, 2 * R)
        .transpose(1, 0, 2).reshape(128, -1),
        dtype=ml_dtypes.bfloat16)

    shared = {
        "wqkvT": AB(in_proj_w.T),
        "woT": AB(np.asarray(out_proj_w).T),
        "vpT": AB(np.asarray(Vp_w).T),
        "qkp_pm": qkp_pm,
        "f1T": AB(f1_eff.T),
        "f2T": AB(np.asarray(ff2_w).T),
        "bcols": bcols,
        "brow": A(brow),
        "lam": A(np.asarray(lam)).reshape(1, 1),
    }
    in_maps = []
    for core in range(8):
        b, h = core // 2, core % 2
        srcb = np.asarray(src[b])
        xTb = srcb.T
        if h == 1:
            # own-query columns first (key order is irrelevant to attention)
            xTb = np.concatenate([xTb[:, SQ:], xTb[:, :SQ]], axis=1)
        m = dict(shared)
        m["xT"] = AB(xTb)
        m["x_own"] = A(srcb[h * SQ:(h + 1) * SQ, :])
        in_maps.append(m)
    return in_maps


def _run(inputs, trace=False):
    if "nc" not in _cached:
        _cached["nc"] = _build()
    nc = _cached["nc"]
    in_maps = _prep_inputs(**inputs)
    res = run_bass_kernel_spmd(nc, in_maps, core_ids=list(range(8)),
                               trace=trace)
    out = np.empty((B, S, D), np.float32)
    for core in range(8):
        b, h = core // 2, core % 2
        out[b, h * SQ:(h + 1) * SQ, :] = res.results[core]["out"]
    return out, res


def kernel(**inputs) -> np.ndarray:
    out, _ = _run(inputs, trace=False)
    return out


# revision 54
# speedup vs baseline: 1.0608x; 1.0608x over previous
"""Trainium2 Bass kernel for the EnhancedEncoderLayer (dense MHA + low-rank
top-k sparse attention + FFN, two layernorms).

Sharding: 8 cores = (batch b in 0..3) x (query-half h in {0,1}). Each core
computes output rows [b, h*512:(h+1)*512, :]. K/V-side projections are
computed redundantly per batch pair (no cross-core communication).

The host permutes src[b].T columns so each core's own query tokens are
columns 0..511 (attention contracts over all keys, so key order is
irrelevant); this keeps the SPMD program identical across cores.

v2 design notes:
- v/vsp projections are x-stationary (lhsT = xT chunk), producing
  token-major Vaug/Vsp directly -- no PE transposes, no ACT copies.
- dense attention is software-pipelined: scores(h) / proj filler /
  ctx(h-1), so ACT exp latency never stalls the in-order PE queue.
- the sparse path runs in bf16: exp writes bf16 psp, the top-k threshold
  bisection scans at 2x DVE rate with 18 iterations, spmm is bf16.
- outproj+spmm+fuse+LN1 run qt-outer so LN1 overlaps matmuls; ff2 is
  qt-outer against an SBUF-resident f2T with LN2+output pipelined per qt.
- ln1 gamma/beta are folded into ff1 weights/bias host-side.
- all host tensors are laid out so every DMA is contiguous per partition.
"""
import sys
import os
import contextlib

for _p in ('/opt/trn_rl_repo',):
    if _p not in sys.path:
        sys.path.insert(0, _p)

import numpy as np
import concourse.bacc as bacc
import concourse.tile as tile
from concourse import mybir
from concourse.bass_utils import run_bass_kernel_spmd
from concourse.masks import make_identity

F32 = mybir.dt.float32
F32R = mybir.dt.float32r
BF16 = mybir.dt.bfloat16
AF = mybir.ActivationFunctionType
OP = mybir.AluOpType

B, S, D, H, R, DFF = 4, 1024, 1024, 16, 64, 4096
DH = D // H          # 64
SQ = S // 2          # 512 own queries per core
KK = max(1, int(S * 0.2))   # 204
KC = D // 128        # 8 contraction chunks over D
FC = DFF // 128      # 32 chunks over DFF
NQT = SQ // 128      # 4 query tiles
NTOK = S // 128      # 8 token tiles
BISECT_ITERS = 19
INV_SQRT = 0.125     # 1/sqrt(DH) == 1/sqrt(R)

_cached = {}


def _build():
    nc = bacc.Bacc()

    def din(name, shape, dt=F32):
        return nc.declare_dram_parameter(name, list(shape), dt, isOutput=False)

    xT = din("xT", [D, S], BF16)      # src[b].T, own-query columns first
    x_own = din("x_own", [SQ, D])     # own rows, token-major
    wqkvT = din("wqkvT", [D, 3 * D], BF16)
    woT = din("woT", [D, D], BF16)
    vpT = din("vpT", [D, D], BF16)
    qkp_pm = din("qkp_pm", [128, KC * 2 * R], BF16)   # partition-major packed
    f1T = din("f1T", [D, DFF], BF16)  # pre-scaled by ln1 gamma
    f2T = din("f2T", [DFF, D], BF16)
    # packed per-partition bias columns: [0:8]=q, [8:16]=k, [16:48]=b1_eff,
    # [48]=bqp (rows 0:64), [49]=bkp (rows 0:64)
    bcols = din("bcols", [128, 50])
    # packed broadcast rows: [bv, bvp, bo, b12, g1, g2, be2]
    brow = din("brow", [1, 7 * D])
    lam = din("lam", [1, 1])
    out = nc.declare_dram_parameter("out", [SQ, D], F32, isOutput=True)

    xT_r = xT.ap().rearrange("(kc p) s -> p kc s", p=128)
    wqkvT_r = wqkvT.ap().rearrange("(kc p) f -> p kc f", p=128)
    woT_r = woT.ap().rearrange("(kc p) f -> p kc f", p=128)
    vpT_r = vpT.ap().rearrange("(kc p) f -> p kc f", p=128)
    qkp_r = qkp_pm.ap().rearrange("p (kc f) -> p kc f", f=2 * R)
    f1T_r = f1T.ap().rearrange("(kc p) f -> p kc f", p=128)
    f2T_r = f2T.ap().rearrange("(kc p) f -> p kc f", p=128)

    with tile.TileContext(nc) as tc:
        est = contextlib.ExitStack()
        with est:
            # ---------------- constants ----------------
            consts = est.enter_context(tc.tile_pool(name="consts", bufs=1))

            ident_f = consts.tile([128, 128], F32, name="ident_f")
            make_identity(nc, ident_f)
            ident_b = consts.tile([128, 128], BF16, name="ident_b")
            nc.vector.tensor_copy(out=ident_b, in_=ident_f)

            eps_t = consts.tile([128, 1], F32, name="eps_t")
            nc.vector.memset(eps_t, 1e-5)
            ones1 = consts.tile([128, 1], F32, name="ones1")
            nc.vector.memset(ones1, 1.0)
            ones1b = consts.tile([128, 1], BF16, name="ones1b")
            nc.vector.memset(ones1b, 1.0)
            ones16b = consts.tile([128, 16], BF16, name="ones16b")
            nc.vector.memset(ones16b, 1.0)

            lam_t = consts.tile([1, 1], F32, name="lam_t")
            nc.sync.dma_start(out=lam_t, in_=lam.ap())
            sg_t = consts.tile([1, 1], F32, name="sg_t")
            nc.scalar.activation(out=sg_t, in_=lam_t, func=AF.Sigmoid)
            sig_bc = consts.tile([128, 1], F32, name="sig_bc")
            nc.gpsimd.partition_broadcast(sig_bc, sg_t)
            oms_bc = consts.tile([128, 1], F32, name="oms_bc")
            nc.vector.tensor_sub(oms_bc, ones1, sig_bc)

            # packed bias columns (one contiguous DMA)
            bcols_t = consts.tile([128, 50], F32, name="bcols_t")
            nc.gpsimd.dma_start(out=bcols_t, in_=bcols.ap())
            bq_c = bcols_t[:, 0:8]
            bk_c = bcols_t[:, 8:16]
            b1_c = bcols_t[:, 16:48]
            bqp_c = bcols_t[0:64, 48:49]
            bkp_c = bcols_t[0:64, 49:50]

            # bisect state
            bis = est.enter_context(tc.tile_pool(name="bis", bufs=1))
            lo = bis.tile([128, NQT], F32, name="lo")
            hi = bis.tile([128, NQT], F32, name="hi")
            mid = bis.tile([128, NQT], F32, name="mid")
            cnts = bis.tile([128, NQT], F32, name="cnts")
            pred = bis.tile([128, NQT], mybir.dt.uint32, name="pred")
            rs_sp = bis.tile([128, NQT], F32, name="rs_sp")
            rcp_sp = bis.tile([128, NQT], F32, name="rcp_sp")

            # long-lived activations
            sp_stack = contextlib.ExitStack()
            sp_pool = sp_stack.enter_context(
                tc.tile_pool(name="sp_pool", bufs=1))
            psp = sp_pool.tile([128, NQT, S], BF16, name="psp")
            kspT = sp_pool.tile([64, S], F32R, name="kspT")
            qspT = sp_pool.tile([64, SQ], F32R, name="qspT")

            av_stack = contextlib.ExitStack()
            av_pool = av_stack.enter_context(
                tc.tile_pool(name="av_pool", bufs=1))
            Vaug = av_pool.tile([128, NTOK, H * (DH + 1)], BF16, name="Vaug")
            Vsp = av_pool.tile([128, NTOK, D], BF16, name="Vsp")
            ctxT = av_pool.tile([128, KC, SQ], BF16, name="ctxT")

            Vaug_h = Vaug.rearrange("p t (h c) -> p t h c", c=DH + 1)
            for t in range(NTOK):
                nc.vector.tensor_copy(out=Vaug_h[:, t, :, DH:DH + 1],
                                      in_=ones16b)

            xot_stack = contextlib.ExitStack()
            xot_pool = xot_stack.enter_context(tc.tile_pool(name="xot_pool",
                                                            bufs=1))
            xot = xot_pool.tile([128, NQT, D], F32, name="xot")

            # out_proj weights (DMA issued later, after the startup crunch)
            wo_stack = contextlib.ExitStack()
            wo_pool = wo_stack.enter_context(
                tc.tile_pool(name="wo_pool", bufs=1))
            woT_s = wo_pool.tile([128, KC, D], BF16, name="woT_s")

            # =========== phase 0/1: input loads, sparse + v projections ====
            xbf_stack = contextlib.ExitStack()
            xbf_pool = xbf_stack.enter_context(
                tc.tile_pool(name="xbf_pool", bufs=1))
            xbf = xbf_pool.tile([128, KC, S], BF16, name="xbf")

            # bisect scratch: lives until after the masking pass
            scr_stack = contextlib.ExitStack()
            scr_pool = scr_stack.enter_context(
                tc.tile_pool(name="scr", bufs=4))

            def bisect_iter():
                # one threshold-bisection step (DVE scans)
                nc.vector.tensor_add(mid, lo, hi)
                nc.vector.tensor_scalar_mul(mid, mid, 0.5)
                for qt in range(NQT):
                    scr = scr_pool.tile([128, S], BF16, name="scr",
                                        tag="scr")
                    nc.vector.scalar_tensor_tensor(
                        out=scr, in0=psp[:, qt, :],
                        scalar=mid[:, qt:qt + 1],
                        in1=ones1b.to_broadcast([128, S]),
                        op0=OP.is_ge, op1=OP.mult,
                        accum_out=cnts[:, qt:qt + 1])
                nc.vector.tensor_scalar(out=pred, in0=cnts,
                                        scalar1=float(KK),
                                        scalar2=None, op0=OP.is_ge)
                nc.vector.copy_predicated(lo, pred, mid)
                nc.vector.tensor_scalar(out=pred, in0=cnts,
                                        scalar1=float(KK),
                                        scalar2=None, op0=OP.is_lt)
                nc.vector.copy_predicated(hi, pred, mid)

            # early broadcast rows: bv, bvp, sig*bo
            early_stack = contextlib.ExitStack()
            early_bc = early_stack.enter_context(
                tc.tile_pool(name="early_bc", bufs=1))
            brow_t = early_bc.tile([1, 3 * D], F32, name="brow_t")
            nc.gpsimd.dma_start(out=brow_t, in_=brow.ap()[:, 0:3 * D])
            bv_bc = early_bc.tile([128, D], F32, name="bv_bc")
            nc.gpsimd.partition_broadcast(bv_bc, brow_t[:, 0:D])
            bvp_bc = early_bc.tile([128, D], F32, name="bvp_bc")
            nc.gpsimd.partition_broadcast(bvp_bc, brow_t[:, D:2 * D])
            bo_sig = early_bc.tile([128, D], F32, name="bo_sig")
            nc.gpsimd.partition_broadcast(bo_sig, brow_t[:, 2 * D:3 * D])
            nc.vector.tensor_scalar_mul(bo_sig, bo_sig, sig_bc)

            with contextlib.ExitStack() as ph0:
                wsp_pool = ph0.enter_context(
                    tc.tile_pool(name="wsp_pool", bufs=1))
                ps_proj = ph0.enter_context(
                    tc.tile_pool(name="ps_proj", bufs=3, space="PSUM"))

                qkpt = wsp_pool.tile([128, KC, 2 * R], BF16, name="qkpt")
                nc.sync.dma_start(out=qkpt, in_=qkp_r)
                qpt = qkpt[:, :, 0:R]
                kpt = qkpt[:, :, R:2 * R]
                for kc in range(KC):
                    eng = nc.scalar if kc % 2 == 0 else nc.sync
                    eng.dma_start(out=xbf[:, kc, :], in_=xT_r[:, kc, :])

                # ---- sparse projections + scores ----
                with nc.named_scope("p0_ksp_qsp"):
                    for nh in range(2):
                        ps = ps_proj.tile([128, 512], F32, name="ps",
                                          tag="ps")
                        for kc in range(KC):
                            nc.tensor.matmul(
                                ps[0:64, :], kpt[:, kc, :],
                                xbf[:, kc, nh * 512:nh * 512 + 512],
                                start=(kc == 0), stop=(kc == KC - 1))
                        nc.scalar.activation(
                            out=kspT[:, nh * 512:nh * 512 + 512],
                            in_=ps[0:64, :], func=AF.Identity, bias=bkp_c,
                            scale=1.0)
                    ps = ps_proj.tile([128, 512], F32, name="ps", tag="ps")
                    for kc in range(KC):
                        nc.tensor.matmul(ps[0:64, :], qpt[:, kc, :],
                                         xbf[:, kc, 0:SQ],
                                         start=(kc == 0), stop=(kc == KC - 1))
                    nc.scalar.activation(out=qspT, in_=ps[0:64, :],
                                         func=AF.Identity, bias=bqp_c,
                                         scale=1.0)

                with nc.named_scope("p2_ssp"):
                    for qt in range(NQT):
                        for nh in range(2):
                            ps = ps_proj.tile([128, 512], F32, name="ps",
                                              tag="ps")
                            nc.tensor.matmul(
                                ps, qspT[:, qt * 128:qt * 128 + 128],
                                kspT[:, nh * 512:nh * 512 + 512],
                                start=True, stop=True)
                            nc.scalar.activation(
                                out=psp[:, qt, nh * 512:nh * 512 + 512],
                                in_=ps, func=AF.Exp, scale=INV_SQRT)

                # own-token residual (+ sig*bo)
                for qt in range(NQT):
                    nc.scalar.dma_start(
                        out=xot[:, qt, :],
                        in_=x_own.ap()[qt * 128:qt * 128 + 128, :])
                    nc.gpsimd.tensor_add(xot[:, qt, :], xot[:, qt, :],
                                         bo_sig)

            # ---- v/vsp x-stationary projections -> token-major ----
            with contextlib.ExitStack() as ph4:
                # reopened weight pool (wv_s, wvp_s still live via av? no --
                # keep them in this scope)
                wv_pool2 = ph4.enter_context(
                    tc.tile_pool(name="wv_pool2", bufs=1))
                wv_s = wv_pool2.tile([128, KC, D], BF16, name="wv_s2")
                wvp_s = wv_pool2.tile([128, KC, D], BF16, name="wvp_s2")
                for kc in range(KC):
                    nc.sync.dma_start(out=wv_s[:, kc, :],
                                      in_=wqkvT_r[:, kc, 2 * D:3 * D])
                    nc.sync.dma_start(out=wvp_s[:, kc, :],
                                      in_=vpT_r[:, kc, :])
                ps_v = ph4.enter_context(
                    tc.tile_pool(name="ps_v", bufs=8, space="PSUM"))
                nc.vector.memset(lo, 0.0)
                nc.vector.memset(hi, 16.0)
                with nc.named_scope("p4_v"):
                    for t in range(NTOK):
                        pva0 = ps_v.tile([128, 512], F32, name="pv", tag="pv")
                        pva1 = ps_v.tile([128, 512], F32, name="pv", tag="pv")
                        pvs0 = ps_v.tile([128, 512], F32, name="pv", tag="pv")
                        pvs1 = ps_v.tile([128, 512], F32, name="pv", tag="pv")
                        for kc in range(KC):
                            xck = xbf[:, kc, t * 128:t * 128 + 128]
                            st = (kc == 0)
                            sp = (kc == KC - 1)
                            nc.tensor.matmul(pva0, xck, wv_s[:, kc, 0:512],
                                             start=st, stop=sp)
                            nc.tensor.matmul(pva1, xck, wv_s[:, kc, 512:1024],
                                             start=st, stop=sp)
                            nc.tensor.matmul(pvs0, xck, wvp_s[:, kc, 0:512],
                                             start=st, stop=sp)
                            nc.tensor.matmul(pvs1, xck, wvp_s[:, kc, 512:1024],
                                             start=st, stop=sp)
                        nc.vector.tensor_add(
                            Vaug_h[:, t, 0:8, 0:DH], pva0, bv_bc[:, 0:512])
                        nc.vector.tensor_add(
                            Vaug_h[:, t, 8:16, 0:DH], pva1, bv_bc[:, 512:1024])
                        nc.vector.tensor_add(
                            Vsp[:, t, 0:512], pvs0, bvp_bc[:, 0:512])
                        nc.vector.tensor_add(
                            Vsp[:, t, 512:1024], pvs1, bvp_bc[:, 512:1024])
                        bisect_iter()
            early_stack.close()   # free bv_bc, bvp_bc, bo_sig

            # out_proj weights resident (used in p6)
            nc.sync.dma_start(out=woT_s, in_=woT_r)

            # ======== phase 5: k/q projections + pipelined attention =======
            kq_stack = contextlib.ExitStack()
            kq_pool = kq_stack.enter_context(
                tc.tile_pool(name="kq_pool", bufs=1))
            kT = kq_pool.tile([128, KC, S], BF16, name="kT")
            qT = kq_pool.tile([128, KC, SQ], BF16, name="qT")
            with contextlib.ExitStack() as ph5:
                wstr = ph5.enter_context(tc.tile_pool(name="wstr", bufs=3))
                pt_pool = ph5.enter_context(
                    tc.tile_pool(name="pt_pool", bufs=16))
                rc_pool = ph5.enter_context(
                    tc.tile_pool(name="rc_pool", bufs=2))
                ps_kq = ph5.enter_context(
                    tc.tile_pool(name="ps_kq", bufs=2, space="PSUM"))
                ps_attn = ph5.enter_context(
                    tc.tile_pool(name="ps_attn", bufs=2, space="PSUM"))
                ps_ctx = ph5.enter_context(
                    tc.tile_pool(name="ps_ctx", bufs=2, space="PSUM"))

                pts = {}
                pctxs = {}
                wkq = {}

                def proj_w(ft):
                    wk = wstr.tile([128, KC, 128], BF16, name="wk", tag="wk")
                    nc.sync.dma_start(
                        out=wk,
                        in_=wqkvT_r[:, :, D + ft * 128:D + ft * 128 + 128])
                    wq = wstr.tile([128, KC, 128], BF16, name="wq", tag="wq")
                    nc.sync.dma_start(
                        out=wq, in_=wqkvT_r[:, :, ft * 128:ft * 128 + 128])
                    wkq[ft] = (wk, wq)

                def proj_k(ft, nh):
                    ps = ps_kq.tile([128, 512], F32, name="pkq", tag="pkq")
                    for kc in range(KC):
                        nc.tensor.matmul(
                            ps, wkq[ft][0][:, kc, :],
                            xbf[:, kc, nh * 512:nh * 512 + 512],
                            start=(kc == 0), stop=(kc == KC - 1))
                    nc.vector.tensor_scalar(
                        out=kT[:, ft, nh * 512:nh * 512 + 512], in0=ps,
                        scalar1=bk_c[:, ft:ft + 1], scalar2=None, op0=OP.add)

                def proj_q(ft):
                    ps = ps_kq.tile([128, 512], F32, name="pkq", tag="pkq")
                    for kc in range(KC):
                        nc.tensor.matmul(ps, wkq[ft][1][:, kc, :],
                                         xbf[:, kc, 0:SQ],
                                         start=(kc == 0), stop=(kc == KC - 1))
                    nc.vector.tensor_scalar(
                        out=qT[:, ft, :], in0=ps,
                        scalar1=bq_c[:, ft:ft + 1], scalar2=None, op0=OP.add)
                    del wkq[ft]

                def scores_group(h0, t2):
                    # heads h0 (rows 0:64) / h0+1 (rows 64:128) issue
                    # interleaved on disjoint PE row groups; each psum tile
                    # spans 2 banks so one wide exp covers 2 token tiles
                    ft = h0 // 2
                    tiles = {}
                    for hh in (h0, h0 + 1):
                        tiles[hh] = ps_attn.tile([128, 1024], F32,
                                                 name="ps_s", tag="ps_s")
                    for tt in range(2):
                        t = 2 * t2 + tt
                        for hh in (h0, h0 + 1):
                            po = 64 * (hh % 2)
                            nc.tensor.matmul(
                                tiles[hh][:, tt * 512:tt * 512 + 512],
                                kT[po:po + 64, ft, t * 128:t * 128 + 128],
                                qT[po:po + 64, ft, :], start=True, stop=True)
                    for hh in (h0, h0 + 1):
                        pt = pt_pool.tile([128, 1024], BF16, name="pT",
                                          tag="pT")
                        nc.scalar.activation(out=pt, in_=tiles[hh],
                                             func=AF.Exp, scale=INV_SQRT)
                        pts.setdefault(hh, []).append(pt)

                def ctx_chunk(hh, half):
                    po = 64 * (hh % 2)
                    ft = hh // 2
                    if half == 0:
                        pctxs[hh] = ps_ctx.tile([128, 512], F32, name="ps_c",
                                                tag="ps_c")
                    pctx = pctxs[hh]
                    for t2 in (2 * half, 2 * half + 1):
                        for tt in range(2):
                            t = 2 * t2 + tt
                            nc.tensor.matmul(
                                pctx[0:65, :], Vaug_h[:, t, hh, 0:DH + 1],
                                pts[hh][t2][:, tt * 512:tt * 512 + 512],
                                start=(t == 0), stop=(t == NTOK - 1))
                    if half == 1:
                        rsr = rc_pool.tile([1, 512], F32, name="rsr",
                                           tag="rsr")
                        nc.vector.tensor_copy(out=rsr, in_=pctx[64:65, :])
                        rch = rc_pool.tile([1, 512], F32, name="rch",
                                           tag="rch")
                        nc.vector.reciprocal_approx_fast(out=rch, in_=rsr)
                        rb = rc_pool.tile([64, 512], F32, name="rb",
                                          tag="rb")
                        nc.gpsimd.partition_broadcast(rb, rch)
                        nc.vector.tensor_mul(out=ctxT[po:po + 64, ft, :],
                                             in0=pctx[0:64, :], in1=rb)
                        del pts[hh]
                        del pctxs[hh]

                with nc.named_scope("p5_kq_attn"):
                    proj_w(0)
                    proj_w(1)
                    proj_k(0, 0)
                    proj_k(0, 1)
                    proj_q(0)
                    proj_k(1, 0)
                    proj_k(1, 1)
                    proj_q(1)
                    bisect_iter()
                    for p in range(H // 2):
                        h0 = 2 * p
                        if p + 2 < KC:
                            proj_w(p + 2)
                            proj_k(p + 2, 0)
                        scores_group(h0, 0)
                        if p >= 1:
                            ctx_chunk(h0 - 2, 0)
                        if p + 2 < KC:
                            proj_k(p + 2, 1)
                        scores_group(h0, 1)
                        if p >= 1:
                            ctx_chunk(h0 - 2, 1)
                        if p + 2 < KC:
                            proj_q(p + 2)
                        scores_group(h0, 2)
                        if p >= 1:
                            ctx_chunk(h0 - 1, 0)
                        scores_group(h0, 3)
                        if p >= 1:
                            ctx_chunk(h0 - 1, 1)
                        bisect_iter()
                    for half in range(2):
                        ctx_chunk(H - 2, half)
                        ctx_chunk(H - 1, half)

                # final masking + renorm scale for the sparse path
                with nc.named_scope("p3_mask"):
                    for qt in range(NQT):
                        nc.vector.scalar_tensor_tensor(
                            out=psp[:, qt, :], in0=psp[:, qt, :],
                            scalar=lo[:, qt:qt + 1],
                            in1=psp[:, qt, :], op0=OP.is_ge, op1=OP.mult,
                            accum_out=rs_sp[:, qt:qt + 1])
                    nc.vector.tensor_scalar(out=rs_sp, in0=rs_sp,
                                            scalar1=1e-9, scalar2=None,
                                            op0=OP.add)
                    nc.vector.reciprocal(rcp_sp, rs_sp)
                    nc.vector.tensor_scalar_mul(rcp_sp, rcp_sp, oms_bc)

            kq_stack.close()    # free kT, qT
            scr_stack.close()
            xbf_stack.close()   # free xbf

            # ========= phase 6: outproj + spmm + fuse + LN1 (qt-outer) =====
            # late broadcast rows: b12, g1, g2, be2 (right-side stack)
            late_bc = est.enter_context(
                tc.tile_pool(name="late_bc", bufs=1, side="right"))
            brow_t2 = late_bc.tile([1, 4 * D], F32, name="brow_t2")
            nc.gpsimd.dma_start(out=brow_t2, in_=brow.ap()[:, 3 * D:7 * D])
            b12_bc = late_bc.tile([128, D], F32, name="b12_bc")
            g1_bc = late_bc.tile([128, D], F32, name="g1_bc")
            g2_bc = late_bc.tile([128, D], F32, name="g2_bc")
            be2_bc = late_bc.tile([128, D], F32, name="be2_bc")
            for i, t_bc in enumerate([b12_bc, g1_bc, g2_bc, be2_bc]):
                nc.gpsimd.partition_broadcast(
                    t_bc, brow_t2[:, i * D:(i + 1) * D])

            fse = est.enter_context(tc.tile_pool(name="fse", bufs=1,
                                                 side="right"))
            x1 = fse.tile([128, NQT, D], F32, name="x1")
            mv2 = fse.tile([128, NQT, 2], F32, name="mv2")
            stats = fse.tile([128, NQT, 2, 6], F32, name="stats")
            sd = fse.tile([128, NQT], F32, name="sd")
            rstd = fse.tile([128, NQT], F32, name="rstd")

            xln_stack = contextlib.ExitStack()
            xlnT_pool = xln_stack.enter_context(
                tc.tile_pool(name="xlnT_pool", bufs=1, side="right"))
            xlnT = xlnT_pool.tile([128, KC, SQ], BF16, name="xlnT")
            w3_stack = contextlib.ExitStack()
            w3str = w3_stack.enter_context(
                tc.tile_pool(name="w3str", bufs=4, side="right"))

            def w1_chunk(jj):
                wt = w3str.tile([128, KC, 256], BF16, name="w1t", tag="w3")
                eng = nc.scalar if jj % 2 == 0 else nc.sync
                eng.dma_start(out=wt, in_=f1T_r[:, :, jj * 256:jj * 256 + 256])
                return wt

            w1_tiles = {jj: w1_chunk(jj) for jj in range(4)}

            xbf1_stack = contextlib.ExitStack()
            xbf1_pool = xbf1_stack.enter_context(
                tc.tile_pool(name="xbf1_pool", bufs=1, side="right"))
            xbf1 = xbf1_pool.tile([128, NQT, D], BF16, name="xbf1")

            def ln_stats(src_ap, qt):
                for half in range(2):
                    nc.vector.bn_stats(
                        out=stats[:, qt, half, :],
                        in_=src_ap[:, half * 512:half * 512 + 512])
                nc.vector.bn_aggr(out=mv2[:, qt, :], in_=stats[:, qt])
                nc.scalar.activation(out=sd[:, qt:qt + 1],
                                     in_=mv2[:, qt, 1:2], func=AF.Sqrt,
                                     bias=eps_t, scale=1.0)
                nc.vector.reciprocal(rstd[:, qt:qt + 1], sd[:, qt:qt + 1])

            with contextlib.ExitStack() as ph6:
                pm_pool = ph6.enter_context(tc.tile_pool(name="pm_pool",
                                                         bufs=2))
                ps_o = ph6.enter_context(
                    tc.tile_pool(name="ps_o", bufs=3, space="PSUM"))
                ps_sp = ph6.enter_context(
                    tc.tile_pool(name="ps_sp", bufs=3, space="PSUM"))
                ps_tr = ph6.enter_context(
                    tc.tile_pool(name="ps_tr", bufs=2, space="PSUM"))
                def xln_transpose(qt):
                    # transpose normalized qt block for ff1 (lagged one qt
                    # so the PE never waits on LN1's DVE chain)
                    qc = slice(qt * 128, qt * 128 + 128)
                    for fc in range(KC):
                        pst = ps_tr.tile([128, 128], BF16, name="pst",
                                         tag="pst")
                        nc.tensor.transpose(
                            pst, xbf1[:, qt, fc * 128:fc * 128 + 128],
                            ident_b)
                        nc.vector.tensor_copy(out=xlnT[:, fc, qc],
                                              in_=pst)

                with nc.named_scope("p6_fuse"):
                    for qt in range(NQT):
                        qc = slice(qt * 128, qt * 128 + 128)
                        # out_proj (2 halves, ctxT-stationary)
                        po0 = ps_o.tile([128, 512], F32, name="po", tag="po")
                        po1 = ps_o.tile([128, 512], F32, name="po", tag="po")
                        for kc in range(KC):
                            st, sp = (kc == 0), (kc == KC - 1)
                            nc.tensor.matmul(po0, ctxT[:, kc, qc],
                                             woT_s[:, kc, 0:512],
                                             start=st, stop=sp)
                            nc.tensor.matmul(po1, ctxT[:, kc, qc],
                                             woT_s[:, kc, 512:1024],
                                             start=st, stop=sp)
                        # masked-p transposes for this qt
                        pmt = pm_pool.tile([128, NTOK, 128], BF16, name="pmt",
                                           tag="pmt")
                        for t in range(NTOK):
                            pst = ps_tr.tile([128, 128], BF16, name="pst",
                                             tag="pst")
                            nc.tensor.transpose(
                                pst, psp[:, qt, t * 128:t * 128 + 128],
                                ident_b)
                            nc.vector.tensor_copy(out=pmt[:, t, :], in_=pst)
                        # spmm (2 halves)
                        sp0 = ps_sp.tile([128, 512], F32, name="psp2",
                                         tag="psp2")
                        sp1 = ps_sp.tile([128, 512], F32, name="psp2",
                                         tag="psp2")
                        for t in range(NTOK):
                            st, spl = (t == 0), (t == NTOK - 1)
                            nc.tensor.matmul(sp0, pmt[:, t, :],
                                             Vsp[:, t, 0:512],
                                             start=st, stop=spl)
                            nc.tensor.matmul(sp1, pmt[:, t, :],
                                             Vsp[:, t, 512:1024],
                                             start=st, stop=spl)
                        if qt >= 1:
                            xln_transpose(qt - 1)
                        # fuse on DVE: x1 = sig*dense + rcp*spmm + xot
                        xq = x1[:, qt, :]
                        nc.vector.tensor_scalar(
                            out=xq[:, 0:512], in0=po0, scalar1=sig_bc,
                            scalar2=None, op0=OP.mult)
                        nc.vector.tensor_scalar(
                            out=xq[:, 512:1024], in0=po1, scalar1=sig_bc,
                            scalar2=None, op0=OP.mult)
                        nc.vector.tensor_add(xq, xq, xot[:, qt, :])
                        nc.vector.scalar_tensor_tensor(
                            out=xq[:, 0:512], in0=sp0,
                            scalar=rcp_sp[:, qt:qt + 1],
                            in1=xq[:, 0:512], op0=OP.mult, op1=OP.add)
                        nc.vector.scalar_tensor_tensor(
                            out=xq[:, 512:1024], in0=sp1,
                            scalar=rcp_sp[:, qt:qt + 1],
                            in1=xq[:, 512:1024], op0=OP.mult, op1=OP.add)
                        # LN1 (keep x1 raw f32 for the ff2 residual)
                        ln_stats(xq, qt)
                        nc.vector.tensor_scalar(
                            out=xbf1[:, qt, :], in0=xq,
                            scalar1=mv2[:, qt, 0:1],
                            scalar2=rstd[:, qt:qt + 1],
                            op0=OP.subtract, op1=OP.mult)
                    xln_transpose(NQT - 1)

            xbf1_stack.close()
            wo_stack.close()
            xot_stack.close()
            av_stack.close()   # free Vaug, Vsp, ctxT
            sp_stack.close()   # free psp, kspT, qspT

            # f2T resident for qt-outer ff2 (chunk DMAs spread through ff1)
            f2_stack = contextlib.ExitStack()
            f2_pool = f2_stack.enter_context(
                tc.tile_pool(name="f2_pool", bufs=1))
            f2_s = f2_pool.tile([128, FC, D], BF16, name="f2_s")

            # xg = xhat*g1 + (be1+b2), computed on DVE during ff1
            xg = fse.tile([128, NQT, D], F32, name="xg")

            # ============ ff1 + relu ============
            h1_stack = contextlib.ExitStack()
            h1_pool = h1_stack.enter_context(
                tc.tile_pool(name="h1_pool", bufs=1))
            h1T = h1_pool.tile([128, FC, SQ], BF16, name="h1T")
            with contextlib.ExitStack() as ph9:
                ps_f1 = ph9.enter_context(
                    tc.tile_pool(name="ps_f1", bufs=4, space="PSUM"))
                with nc.named_scope("p9_ff1"):
                    for jj in range(16):
                        wt = w1_tiles.pop(jj)
                        if jj + 4 < 16:
                            w1_tiles[jj + 4] = w1_chunk(jj + 4)
                        for kc2 in range(2):
                            nc.gpsimd.dma_start(
                                out=f2_s[:, jj * 2 + kc2, :],
                                in_=f2T_r[:, jj * 2 + kc2, :])
                        for fi in range(2):
                            dft = jj * 2 + fi
                            ps = ps_f1.tile([128, 512], F32, name="ps_f",
                                            tag="ps_f")
                            for kc in range(KC):
                                nc.tensor.matmul(
                                    ps, wt[:, kc, fi * 128:fi * 128 + 128],
                                    xlnT[:, kc, :],
                                    start=(kc == 0), stop=(kc == KC - 1))
                            nc.scalar.activation(
                                out=h1T[:, dft, :], in_=ps, func=AF.Relu,
                                bias=b1_c[:, dft:dft + 1], scale=1.0)
                        if jj < 2 * NQT and jj % 2 == 1:
                            # xg for qt = jj//2, hidden under ff1
                            qt = jj // 2
                            nc.vector.tensor_scalar(
                                out=xg[:, qt, :], in0=x1[:, qt, :],
                                scalar1=mv2[:, qt, 0:1],
                                scalar2=rstd[:, qt:qt + 1],
                                op0=OP.subtract, op1=OP.mult)
                            nc.vector.tensor_mul(xg[:, qt, :], xg[:, qt, :],
                                                 g1_bc)
                            nc.vector.tensor_add(xg[:, qt, :], xg[:, qt, :],
                                                 b12_bc)
            w3_stack.close()
            xln_stack.close()

            # ============ ff2 (qt-outer) + residual + LN2 + out ============
            with contextlib.ExitStack() as ph10:
                ps_f2 = ph10.enter_context(
                    tc.tile_pool(name="ps_f2", bufs=4, space="PSUM"))
                ot_pool = ph10.enter_context(
                    tc.tile_pool(name="ot_pool", bufs=2))
                with nc.named_scope("p10_ff2"):
                    for qt in range(NQT):
                        qc = slice(qt * 128, qt * 128 + 128)
                        pg0 = ps_f2.tile([128, 512], F32, name="pg", tag="pg")
                        pg1 = ps_f2.tile([128, 512], F32, name="pg", tag="pg")
                        for kc in range(FC):
                            st, sp = (kc == 0), (kc == FC - 1)
                            nc.tensor.matmul(pg0, h1T[:, kc, qc],
                                             f2_s[:, kc, 0:512],
                                             start=st, stop=sp)
                            nc.tensor.matmul(pg1, h1T[:, kc, qc],
                                             f2_s[:, kc, 512:1024],
                                             start=st, stop=sp)
                        x2 = x1[:, qt, :]
                        nc.vector.tensor_add(x2[:, 0:512], pg0,
                                             xg[:, qt, 0:512])
                        nc.vector.tensor_add(x2[:, 512:1024], pg1,
                                             xg[:, qt, 512:1024])
                        ln_stats(x2, qt)
                        ot = ot_pool.tile([128, D], F32, name="out_t",
                                          tag="out_t")
                        nc.vector.tensor_scalar(
                            out=ot, in0=x2, scalar1=mv2[:, qt, 0:1],
                            scalar2=rstd[:, qt:qt + 1],
                            op0=OP.subtract, op1=OP.mult)
                        nc.vector.tensor_mul(ot, ot, g2_bc)
                        nc.vector.tensor_add(ot, ot, be2_bc)
                        nc.scalar.dma_start(
                            out=out.ap()[qt * 128:qt * 128 + 128, :], in_=ot)
            h1_stack.close()
            f2_stack.close()

    nc.compile()
    return nc


def _prep_inputs(src, in_proj_w, in_proj_b, out_proj_w, out_proj_b,
                 Qp_w, Qp_b, Kp_w, Kp_b, Vp_w, Vp_b, lam,
                 ff1_w, ff1_b, ff2_w, ff2_b, ln1_g, ln1_b, ln2_g, ln2_b):
    import ml_dtypes
    f = np.float32
    A = lambda x: np.ascontiguousarray(x, dtype=f)
    AB = lambda x: np.ascontiguousarray(np.asarray(x, dtype=f),
                                        dtype=ml_dtypes.bfloat16)
    in_proj_w = np.asarray(in_proj_w, dtype=f)
    ff1_w = np.asarray(ff1_w, dtype=f)
    ln1_g = np.asarray(ln1_g, dtype=f)
    ln1_b = np.asarray(ln1_b, dtype=f)
    # fold ln1 gamma into ff1 weights, ln1 beta into ff1 bias
    f1_eff = ff1_w * ln1_g[None, :]            # [DFF, D]
    b1_eff = np.asarray(ff1_b, dtype=f) + ff1_w @ ln1_b

    # packed per-partition bias columns [128, 50]
    bcols = np.zeros((128, 50), dtype=f)
    qk_bias = np.asarray(in_proj_b, dtype=f)
    bcols[:, 0:8] = qk_bias[0:D].reshape(8, 128).T
    bcols[:, 8:16] = qk_bias[D:2 * D].reshape(8, 128).T
    bcols[:, 16:48] = b1_eff.reshape(32, 128).T
    bcols[0:64, 48] = np.asarray(Qp_b, dtype=f)
    bcols[0:64, 49] = np.asarray(Kp_b, dtype=f)

    # packed broadcast rows [1, 7D]: bv, bvp, bo, b12, g1, g2, be2
    brow = np.concatenate([
        qk_bias[2 * D:3 * D],
        np.asarray(Vp_b, dtype=f),
        np.asarray(out_proj_b, dtype=f),
        ln1_b + np.asarray(ff2_b, dtype=f),
        ln1_g,
        np.asarray(ln2_g, dtype=f),
        np.asarray(ln2_b, dtype=f),
    ]).reshape(1, 7 * D)

    # qkp packed partition-major: [128, kc, 2R] contiguous
    qkpT = np.concatenate([np.asarray(Qp_w).T, np.asarray(Kp_w).T], axis=1)
    qkp_pm = np.ascontiguousarray(
        np.asarray(qkpT, dtype=f).reshape(KC, 128, 2 * R)
        .transpose(1, 0, 2).reshape(128, -1),
        dtype=ml_dtypes.bfloat16)

    shared = {
        "wqkvT": AB(in_proj_w.T),
        "woT": AB(np.asarray(out_proj_w).T),
        "vpT": AB(np.asarray(Vp_w).T),
        "qkp_pm": qkp_pm,
        "f1T": AB(f1_eff.T),
        "f2T": AB(np.asarray(ff2_w).T),
        "bcols": bcols,
        "brow": A(brow),
        "lam": A(np.asarray(lam)).reshape(1, 1),
    }
    in_maps = []
    for core in range(8):
        b, h = core // 2, core % 2
        srcb = np.asarray(src[b])
        xTb = srcb.T
        if h == 1:
            # own-query columns first (key order is irrelevant to attention)
            xTb = np.concatenate([xTb[:, SQ:], xTb[:, :SQ]], axis=1)
        m = dict(shared)
        m["xT"] = AB(xTb)
        m["x_own"] = A(srcb[h * SQ:(h + 1) * SQ, :])
        in_maps.append(m)
    return in_maps


def _run(inputs, trace=False):
    if "nc" not in _cached:
        _cached["nc"] = _build()
    nc = _cached["nc"]
    in_maps = _prep_inputs(**inputs)
    res = run_bass_kernel_spmd(nc, in_maps, core_ids=list(range(8)),
                               trace=trace)
    out = np.empty((B, S, D), np.float32)
    for core in range(8):
        b, h = core // 2, core % 2
        out[b, h * SQ:(h + 1) * SQ, :] = res.results[core]["out"]
    return out, res


def kernel(**inputs) -> np.ndarray:
    out, _ = _run(inputs, trace=False)
    return out


# revision 56
# speedup vs baseline: 1.0941x; 1.0314x over previous
"""Trainium2 Bass kernel for the EnhancedEncoderLayer (dense MHA + low-rank
top-k sparse attention + FFN, two layernorms).

Sharding: 8 cores = (batch b in 0..3) x (query-half h in {0,1}). Each core
computes output rows [b, h*512:(h+1)*512, :]. K/V-side projections are
computed redundantly per batch pair (no cross-core communication).

The host permutes src[b].T columns so each core's own query tokens are
columns 0..511 (attention contracts over all keys, so key order is
irrelevant); this keeps the SPMD program identical across cores.

v2 design notes:
- v/vsp projections are x-stationary (lhsT = xT chunk), producing
  token-major Vaug/Vsp directly -- no PE transposes, no ACT copies.
- dense attention is software-pipelined: scores(h) / proj filler /
  ctx(h-1), so ACT exp latency never stalls the in-order PE queue.
- the sparse path runs in bf16: exp writes bf16 psp, the top-k threshold
  bisection scans at 2x DVE rate with 18 iterations, spmm is bf16.
- outproj+spmm+fuse+LN1 run qt-outer so LN1 overlaps matmuls; ff2 is
  qt-outer against an SBUF-resident f2T with LN2+output pipelined per qt.
- ln1 gamma/beta are folded into ff1 weights/bias host-side.
- all host tensors are laid out so every DMA is contiguous per partition.
"""
import sys
import os
import contextlib

for _p in ('/opt/trn_rl_repo',):
    if _p not in sys.path:
        sys.path.insert(0, _p)

import numpy as np
import concourse.bacc as bacc
import concourse.tile as tile
from concourse import mybir
from concourse.bass_utils import run_bass_kernel_spmd
from concourse.masks import make_identity

F32 = mybir.dt.float32
F32R = mybir.dt.float32r
BF16 = mybir.dt.bfloat16
AF = mybir.ActivationFunctionType
OP = mybir.AluOpType

B, S, D, H, R, DFF = 4, 1024, 1024, 16, 64, 4096
DH = D // H          # 64
SQ = S // 2          # 512 own queries per core
KK = max(1, int(S * 0.2))   # 204
KC = D // 128        # 8 contraction chunks over D
FC = DFF // 128      # 32 chunks over DFF
NQT = SQ // 128      # 4 query tiles
NTOK = S // 128      # 8 token tiles
BISECT_ITERS = 19
INV_SQRT = 0.125     # 1/sqrt(DH) == 1/sqrt(R)

_cached = {}


def _build():
    nc = bacc.Bacc()

    def din(name, shape, dt=F32):
        return nc.declare_dram_parameter(name, list(shape), dt, isOutput=False)

    xT = din("xT", [D, S], BF16)      # src[b].T, own-query columns first
    x_own = din("x_own", [SQ, D])     # own rows, token-major
    wqkvT = din("wqkvT", [D, 3 * D], BF16)
    woT = din("woT", [D, D], BF16)
    vpT = din("vpT", [D, D], BF16)
    qkp_pm = din("qkp_pm", [128, KC * 2 * R], BF16)   # partition-major packed
    f1T = din("f1T", [D, DFF], BF16)  # pre-scaled by ln1 gamma
    f2T = din("f2T", [DFF, D], BF16)
    # packed per-partition bias columns: [0:8]=q, [8:16]=k, [16:48]=b1_eff,
    # [48]=bqp (rows 0:64), [49]=bkp (rows 0:64)
    bcols = din("bcols", [128, 50])
    # packed broadcast rows: [bv, bvp, bo, b12, g1, g2, be2]
    brow = din("brow", [1, 7 * D])
    lam = din("lam", [1, 1])
    out = nc.declare_dram_parameter("out", [SQ, D], F32, isOutput=True)

    xT_r = xT.ap().rearrange("(kc p) s -> p kc s", p=128)
    wqkvT_r = wqkvT.ap().rearrange("(kc p) f -> p kc f", p=128)
    woT_r = woT.ap().rearrange("(kc p) f -> p kc f", p=128)
    vpT_r = vpT.ap().rearrange("(kc p) f -> p kc f", p=128)
    qkp_r = qkp_pm.ap().rearrange("p (kc f) -> p kc f", f=2 * R)
    f1T_r = f1T.ap().rearrange("(kc p) f -> p kc f", p=128)
    f2T_r = f2T.ap().rearrange("(kc p) f -> p kc f", p=128)

    with tile.TileContext(nc) as tc:
        est = contextlib.ExitStack()
        with est:
            # ---------------- constants ----------------
            consts = est.enter_context(tc.tile_pool(name="consts", bufs=1))

            ident_f = consts.tile([128, 128], F32, name="ident_f")
            make_identity(nc, ident_f)
            ident_b = consts.tile([128, 128], BF16, name="ident_b")
            nc.vector.tensor_copy(out=ident_b, in_=ident_f)

            eps_t = consts.tile([128, 1], F32, name="eps_t")
            nc.vector.memset(eps_t, 1e-5)
            ones1 = consts.tile([128, 1], F32, name="ones1")
            nc.vector.memset(ones1, 1.0)
            ones1b = consts.tile([128, 1], BF16, name="ones1b")
            nc.vector.memset(ones1b, 1.0)
            ones16b = consts.tile([128, 16], BF16, name="ones16b")
            nc.vector.memset(ones16b, 1.0)

            lam_t = consts.tile([1, 1], F32, name="lam_t")
            nc.sync.dma_start(out=lam_t, in_=lam.ap())
            sg_t = consts.tile([1, 1], F32, name="sg_t")
            nc.scalar.activation(out=sg_t, in_=lam_t, func=AF.Sigmoid)
            sig_bc = consts.tile([128, 1], F32, name="sig_bc")
            nc.gpsimd.partition_broadcast(sig_bc, sg_t)
            oms_bc = consts.tile([128, 1], F32, name="oms_bc")
            nc.vector.tensor_sub(oms_bc, ones1, sig_bc)

            # packed bias columns (one contiguous DMA)
            bcols_t = consts.tile([128, 50], F32, name="bcols_t")
            nc.gpsimd.dma_start(out=bcols_t, in_=bcols.ap())
            bq_c = bcols_t[:, 0:8]
            bk_c = bcols_t[:, 8:16]
            b1_c = bcols_t[:, 16:48]
            bqp_c = bcols_t[0:64, 48:49]
            bkp_c = bcols_t[0:64, 49:50]

            # bisect state
            bis = est.enter_context(tc.tile_pool(name="bis", bufs=1))
            lo = bis.tile([128, NQT], F32, name="lo")
            hi = bis.tile([128, NQT], F32, name="hi")
            mid = bis.tile([128, NQT], F32, name="mid")
            cnts = bis.tile([128, NQT], F32, name="cnts")
            pred = bis.tile([128, NQT], mybir.dt.uint32, name="pred")
            rs_sp = bis.tile([128, NQT], F32, name="rs_sp")
            rcp_sp = bis.tile([128, NQT], F32, name="rcp_sp")

            # long-lived activations
            sp_stack = contextlib.ExitStack()
            sp_pool = sp_stack.enter_context(
                tc.tile_pool(name="sp_pool", bufs=1))
            psp = sp_pool.tile([128, NQT, S], BF16, name="psp")
            kspT = sp_pool.tile([64, S], F32R, name="kspT")
            qspT = sp_pool.tile([64, SQ], F32R, name="qspT")

            av_stack = contextlib.ExitStack()
            av_pool = av_stack.enter_context(
                tc.tile_pool(name="av_pool", bufs=1))
            Vaug = av_pool.tile([128, NTOK, H * (DH + 1)], BF16, name="Vaug")
            Vsp = av_pool.tile([128, NTOK, D], BF16, name="Vsp")
            ctxT = av_pool.tile([128, KC, SQ], BF16, name="ctxT")

            Vaug_h = Vaug.rearrange("p t (h c) -> p t h c", c=DH + 1)
            for t in range(NTOK):
                nc.vector.tensor_copy(out=Vaug_h[:, t, :, DH:DH + 1],
                                      in_=ones16b)

            xot_stack = contextlib.ExitStack()
            xot_pool = xot_stack.enter_context(tc.tile_pool(name="xot_pool",
                                                            bufs=1))
            xot = xot_pool.tile([128, NQT, D], F32, name="xot")

            # out_proj weights (DMA issued later, after the startup crunch)
            wo_stack = contextlib.ExitStack()
            wo_pool = wo_stack.enter_context(
                tc.tile_pool(name="wo_pool", bufs=1))
            woT_s = wo_pool.tile([128, KC, D], BF16, name="woT_s")

            # =========== phase 0/1: input loads, sparse + v projections ====
            xbf_stack = contextlib.ExitStack()
            xbf_pool = xbf_stack.enter_context(
                tc.tile_pool(name="xbf_pool", bufs=1))
            xbf = xbf_pool.tile([128, KC, S], BF16, name="xbf")

            # bisect scratch: lives until after the masking pass
            scr_stack = contextlib.ExitStack()
            scr_pool = scr_stack.enter_context(
                tc.tile_pool(name="scr", bufs=4))

            def bisect_iter():
                # one threshold-bisection step (DVE scans)
                nc.vector.tensor_add(mid, lo, hi)
                nc.vector.tensor_scalar_mul(mid, mid, 0.5)
                for qt in range(NQT):
                    scr = scr_pool.tile([128, S], BF16, name="scr",
                                        tag="scr")
                    nc.vector.scalar_tensor_tensor(
                        out=scr, in0=psp[:, qt, :],
                        scalar=mid[:, qt:qt + 1],
                        in1=ones1b.to_broadcast([128, S]),
                        op0=OP.is_ge, op1=OP.mult,
                        accum_out=cnts[:, qt:qt + 1])
                nc.vector.tensor_scalar(out=pred, in0=cnts,
                                        scalar1=float(KK),
                                        scalar2=None, op0=OP.is_ge)
                nc.vector.copy_predicated(lo, pred, mid)
                nc.vector.tensor_scalar(out=pred, in0=cnts,
                                        scalar1=float(KK),
                                        scalar2=None, op0=OP.is_lt)
                nc.vector.copy_predicated(hi, pred, mid)

            # early broadcast rows: bv, bvp, sig*bo
            early_stack = contextlib.ExitStack()
            early_bc = early_stack.enter_context(
                tc.tile_pool(name="early_bc", bufs=1))
            brow_t = early_bc.tile([1, 3 * D], F32, name="brow_t")
            nc.gpsimd.dma_start(out=brow_t, in_=brow.ap()[:, 0:3 * D])
            bv_bc = early_bc.tile([128, D], F32, name="bv_bc")
            nc.gpsimd.partition_broadcast(bv_bc, brow_t[:, 0:D])
            bvp_bc = early_bc.tile([128, D], F32, name="bvp_bc")
            nc.gpsimd.partition_broadcast(bvp_bc, brow_t[:, D:2 * D])
            bo_sig = early_bc.tile([128, D], F32, name="bo_sig")
            nc.gpsimd.partition_broadcast(bo_sig, brow_t[:, 2 * D:3 * D])
            nc.vector.tensor_scalar_mul(bo_sig, bo_sig, sig_bc)

            with contextlib.ExitStack() as ph0:
                wsp_pool = ph0.enter_context(
                    tc.tile_pool(name="wsp_pool", bufs=1))
                ps_proj = ph0.enter_context(
                    tc.tile_pool(name="ps_proj", bufs=3, space="PSUM"))

                qkpt = wsp_pool.tile([128, KC, 2 * R], BF16, name="qkpt")
                nc.sync.dma_start(out=qkpt, in_=qkp_r)
                qpt = qkpt[:, :, 0:R]
                kpt = qkpt[:, :, R:2 * R]
                for kc in range(KC):
                    eng = nc.scalar if kc % 2 == 0 else nc.sync
                    eng.dma_start(out=xbf[:, kc, :], in_=xT_r[:, kc, :])

                # ---- sparse projections + scores ----
                with nc.named_scope("p0_ksp_qsp"):
                    for nh in range(2):
                        ps = ps_proj.tile([128, 512], F32, name="ps",
                                          tag="ps")
                        for kc in range(KC):
                            nc.tensor.matmul(
                                ps[0:64, :], kpt[:, kc, :],
                                xbf[:, kc, nh * 512:nh * 512 + 512],
                                start=(kc == 0), stop=(kc == KC - 1))
                        nc.scalar.activation(
                            out=kspT[:, nh * 512:nh * 512 + 512],
                            in_=ps[0:64, :], func=AF.Identity, bias=bkp_c,
                            scale=1.0)
                    ps = ps_proj.tile([128, 512], F32, name="ps", tag="ps")
                    for kc in range(KC):
                        nc.tensor.matmul(ps[0:64, :], qpt[:, kc, :],
                                         xbf[:, kc, 0:SQ],
                                         start=(kc == 0), stop=(kc == KC - 1))
                    nc.scalar.activation(out=qspT, in_=ps[0:64, :],
                                         func=AF.Identity, bias=bqp_c,
                                         scale=1.0)

                with nc.named_scope("p2_ssp"):
                    for qt in range(NQT):
                        for nh in range(2):
                            ps = ps_proj.tile([128, 512], F32, name="ps",
                                              tag="ps")
                            nc.tensor.matmul(
                                ps, qspT[:, qt * 128:qt * 128 + 128],
                                kspT[:, nh * 512:nh * 512 + 512],
                                start=True, stop=True)
                            nc.scalar.activation(
                                out=psp[:, qt, nh * 512:nh * 512 + 512],
                                in_=ps, func=AF.Exp, scale=INV_SQRT)

                # own-token residual (+ sig*bo)
                for qt in range(NQT):
                    nc.scalar.dma_start(
                        out=xot[:, qt, :],
                        in_=x_own.ap()[qt * 128:qt * 128 + 128, :])
                    nc.gpsimd.tensor_add(xot[:, qt, :], xot[:, qt, :],
                                         bo_sig)

            # ---- v/vsp x-stationary projections -> token-major ----
            with contextlib.ExitStack() as ph4:
                # reopened weight pool (wv_s, wvp_s still live via av? no --
                # keep them in this scope)
                wv_pool2 = ph4.enter_context(
                    tc.tile_pool(name="wv_pool2", bufs=1))
                wv_s = wv_pool2.tile([128, KC, D], BF16, name="wv_s2")
                wvp_s = wv_pool2.tile([128, KC, D], BF16, name="wvp_s2")
                for kc in range(KC):
                    nc.sync.dma_start(out=wv_s[:, kc, :],
                                      in_=wqkvT_r[:, kc, 2 * D:3 * D])
                    nc.sync.dma_start(out=wvp_s[:, kc, :],
                                      in_=vpT_r[:, kc, :])
                ps_v = ph4.enter_context(
                    tc.tile_pool(name="ps_v", bufs=8, space="PSUM"))
                nc.vector.memset(lo, 0.0)
                nc.vector.memset(hi, 16.0)
                with nc.named_scope("p4_v"):
                    for t in range(NTOK):
                        pva0 = ps_v.tile([128, 512], F32, name="pv", tag="pv")
                        pva1 = ps_v.tile([128, 512], F32, name="pv", tag="pv")
                        pvs0 = ps_v.tile([128, 512], F32, name="pv", tag="pv")
                        pvs1 = ps_v.tile([128, 512], F32, name="pv", tag="pv")
                        for kc in range(KC):
                            xck = xbf[:, kc, t * 128:t * 128 + 128]
                            st = (kc == 0)
                            sp = (kc == KC - 1)
                            nc.tensor.matmul(pva0, xck, wv_s[:, kc, 0:512],
                                             start=st, stop=sp)
                            nc.tensor.matmul(pva1, xck, wv_s[:, kc, 512:1024],
                                             start=st, stop=sp)
                            nc.tensor.matmul(pvs0, xck, wvp_s[:, kc, 0:512],
                                             start=st, stop=sp)
                            nc.tensor.matmul(pvs1, xck, wvp_s[:, kc, 512:1024],
                                             start=st, stop=sp)
                        nc.vector.tensor_add(
                            Vaug_h[:, t, 0:8, 0:DH], pva0, bv_bc[:, 0:512])
                        nc.vector.tensor_add(
                            Vaug_h[:, t, 8:16, 0:DH], pva1, bv_bc[:, 512:1024])
                        nc.vector.tensor_add(
                            Vsp[:, t, 0:512], pvs0, bvp_bc[:, 0:512])
                        nc.vector.tensor_add(
                            Vsp[:, t, 512:1024], pvs1, bvp_bc[:, 512:1024])
                        bisect_iter()
            early_stack.close()   # free bv_bc, bvp_bc, bo_sig

            # out_proj weights resident (used in p6; scalar queue is idle)
            nc.scalar.dma_start(out=woT_s, in_=woT_r)

            # ======== phase 5: k/q projections + pipelined attention =======
            kq_stack = contextlib.ExitStack()
            kq_pool = kq_stack.enter_context(
                tc.tile_pool(name="kq_pool", bufs=1))
            kT = kq_pool.tile([128, KC, S], BF16, name="kT")
            qT = kq_pool.tile([128, KC, SQ], BF16, name="qT")
            with contextlib.ExitStack() as ph5:
                wstr = ph5.enter_context(tc.tile_pool(name="wstr", bufs=3))
                pt_pool = ph5.enter_context(
                    tc.tile_pool(name="pt_pool", bufs=16))
                rc_pool = ph5.enter_context(
                    tc.tile_pool(name="rc_pool", bufs=2))
                ps_kq = ph5.enter_context(
                    tc.tile_pool(name="ps_kq", bufs=2, space="PSUM"))
                ps_attn = ph5.enter_context(
                    tc.tile_pool(name="ps_attn", bufs=2, space="PSUM"))
                ps_ctx = ph5.enter_context(
                    tc.tile_pool(name="ps_ctx", bufs=2, space="PSUM"))

                pts = {}
                pctxs = {}
                wkq = {}

                def proj_w(ft):
                    wk = wstr.tile([128, KC, 128], BF16, name="wk", tag="wk")
                    nc.sync.dma_start(
                        out=wk,
                        in_=wqkvT_r[:, :, D + ft * 128:D + ft * 128 + 128])
                    wq = wstr.tile([128, KC, 128], BF16, name="wq", tag="wq")
                    nc.sync.dma_start(
                        out=wq, in_=wqkvT_r[:, :, ft * 128:ft * 128 + 128])
                    wkq[ft] = (wk, wq)

                def proj_k(ft, nh):
                    ps = ps_kq.tile([128, 512], F32, name="pkq", tag="pkq")
                    for kc in range(KC):
                        nc.tensor.matmul(
                            ps, wkq[ft][0][:, kc, :],
                            xbf[:, kc, nh * 512:nh * 512 + 512],
                            start=(kc == 0), stop=(kc == KC - 1))
                    nc.vector.tensor_scalar(
                        out=kT[:, ft, nh * 512:nh * 512 + 512], in0=ps,
                        scalar1=bk_c[:, ft:ft + 1], scalar2=None, op0=OP.add)

                def proj_q(ft):
                    ps = ps_kq.tile([128, 512], F32, name="pkq", tag="pkq")
                    for kc in range(KC):
                        nc.tensor.matmul(ps, wkq[ft][1][:, kc, :],
                                         xbf[:, kc, 0:SQ],
                                         start=(kc == 0), stop=(kc == KC - 1))
                    nc.vector.tensor_scalar(
                        out=qT[:, ft, :], in0=ps,
                        scalar1=bq_c[:, ft:ft + 1], scalar2=None, op0=OP.add)
                    del wkq[ft]

                def scores_group(h0, t2):
                    # heads h0 (rows 0:64) / h0+1 (rows 64:128) issue
                    # interleaved on disjoint PE row groups; each psum tile
                    # spans 2 banks so one wide exp covers 2 token tiles
                    ft = h0 // 2
                    tiles = {}
                    for hh in (h0, h0 + 1):
                        tiles[hh] = ps_attn.tile([128, 1024], F32,
                                                 name="ps_s", tag="ps_s")
                    for tt in range(2):
                        t = 2 * t2 + tt
                        for hh in (h0, h0 + 1):
                            po = 64 * (hh % 2)
                            nc.tensor.matmul(
                                tiles[hh][:, tt * 512:tt * 512 + 512],
                                kT[po:po + 64, ft, t * 128:t * 128 + 128],
                                qT[po:po + 64, ft, :], start=True, stop=True)
                    for hh in (h0, h0 + 1):
                        pt = pt_pool.tile([128, 1024], BF16, name="pT",
                                          tag="pT")
                        nc.scalar.activation(out=pt, in_=tiles[hh],
                                             func=AF.Exp, scale=INV_SQRT)
                        pts.setdefault(hh, []).append(pt)

                def ctx_chunk(hh, half):
                    po = 64 * (hh % 2)
                    ft = hh // 2
                    if half == 0:
                        pctxs[hh] = ps_ctx.tile([128, 512], F32, name="ps_c",
                                                tag="ps_c")
                    pctx = pctxs[hh]
                    for t2 in (2 * half, 2 * half + 1):
                        for tt in range(2):
                            t = 2 * t2 + tt
                            nc.tensor.matmul(
                                pctx[0:65, :], Vaug_h[:, t, hh, 0:DH + 1],
                                pts[hh][t2][:, tt * 512:tt * 512 + 512],
                                start=(t == 0), stop=(t == NTOK - 1))
                    if half == 1:
                        rsr = rc_pool.tile([1, 512], F32, name="rsr",
                                           tag="rsr")
                        nc.vector.tensor_copy(out=rsr, in_=pctx[64:65, :])
                        rch = rc_pool.tile([1, 512], F32, name="rch",
                                           tag="rch")
                        nc.vector.reciprocal_approx_fast(out=rch, in_=rsr)
                        rb = rc_pool.tile([64, 512], F32, name="rb",
                                          tag="rb")
                        nc.gpsimd.partition_broadcast(rb, rch)
                        nc.vector.tensor_mul(out=ctxT[po:po + 64, ft, :],
                                             in0=pctx[0:64, :], in1=rb)
                        del pts[hh]
                        del pctxs[hh]

                with nc.named_scope("p5_kq_attn"):
                    proj_w(0)
                    proj_w(1)
                    proj_k(0, 0)
                    proj_k(0, 1)
                    proj_q(0)
                    proj_k(1, 0)
                    proj_k(1, 1)
                    proj_q(1)
                    bisect_iter()
                    for p in range(H // 2):
                        h0 = 2 * p
                        if p + 2 < KC:
                            proj_w(p + 2)
                            proj_k(p + 2, 0)
                        scores_group(h0, 0)
                        if p >= 1:
                            ctx_chunk(h0 - 2, 0)
                        if p + 2 < KC:
                            proj_k(p + 2, 1)
                        scores_group(h0, 1)
                        if p >= 1:
                            ctx_chunk(h0 - 2, 1)
                        if p + 2 < KC:
                            proj_q(p + 2)
                        scores_group(h0, 2)
                        if p >= 1:
                            ctx_chunk(h0 - 1, 0)
                        scores_group(h0, 3)
                        if p >= 1:
                            ctx_chunk(h0 - 1, 1)
                        if p < 6:
                            bisect_iter()
                    for half in range(2):
                        ctx_chunk(H - 2, half)
                        ctx_chunk(H - 1, half)

                # final masking + renorm scale for the sparse path
                with nc.named_scope("p3_mask"):
                    for qt in range(NQT):
                        nc.vector.scalar_tensor_tensor(
                            out=psp[:, qt, :], in0=psp[:, qt, :],
                            scalar=lo[:, qt:qt + 1],
                            in1=psp[:, qt, :], op0=OP.is_ge, op1=OP.mult,
                            accum_out=rs_sp[:, qt:qt + 1])
                    nc.vector.tensor_scalar(out=rs_sp, in0=rs_sp,
                                            scalar1=1e-9, scalar2=None,
                                            op0=OP.add)
                    nc.vector.reciprocal(rcp_sp, rs_sp)
                    nc.vector.tensor_scalar_mul(rcp_sp, rcp_sp, oms_bc)

            kq_stack.close()    # free kT, qT
            scr_stack.close()
            xbf_stack.close()   # free xbf

            # ========= phase 6: outproj + spmm + fuse + LN1 (qt-outer) =====
            # late broadcast rows: b12, g1, g2, be2 (right-side stack)
            late_bc = est.enter_context(
                tc.tile_pool(name="late_bc", bufs=1, side="right"))
            brow_t2 = late_bc.tile([1, 4 * D], F32, name="brow_t2")
            nc.gpsimd.dma_start(out=brow_t2, in_=brow.ap()[:, 3 * D:7 * D])
            b12_bc = late_bc.tile([128, D], F32, name="b12_bc")
            g1_bc = late_bc.tile([128, D], F32, name="g1_bc")
            g2_bc = late_bc.tile([128, D], F32, name="g2_bc")
            be2_bc = late_bc.tile([128, D], F32, name="be2_bc")
            for i, t_bc in enumerate([b12_bc, g1_bc, g2_bc, be2_bc]):
                nc.gpsimd.partition_broadcast(
                    t_bc, brow_t2[:, i * D:(i + 1) * D])

            fse = est.enter_context(tc.tile_pool(name="fse", bufs=1,
                                                 side="right"))
            x1 = fse.tile([128, NQT, D], F32, name="x1")
            mv2 = fse.tile([128, NQT, 2], F32, name="mv2")
            stats = fse.tile([128, NQT, 2, 6], F32, name="stats")
            sd = fse.tile([128, NQT], F32, name="sd")
            rstd = fse.tile([128, NQT], F32, name="rstd")

            xln_stack = contextlib.ExitStack()
            xlnT_pool = xln_stack.enter_context(
                tc.tile_pool(name="xlnT_pool", bufs=1, side="right"))
            xlnT = xlnT_pool.tile([128, KC, SQ], BF16, name="xlnT")
            w3_stack = contextlib.ExitStack()
            w3str = w3_stack.enter_context(
                tc.tile_pool(name="w3str", bufs=4, side="right"))

            def w1_chunk(jj):
                wt = w3str.tile([128, KC, 256], BF16, name="w1t", tag="w3")
                eng = nc.scalar if jj % 2 == 0 else nc.sync
                eng.dma_start(out=wt, in_=f1T_r[:, :, jj * 256:jj * 256 + 256])
                return wt

            w1_tiles = {jj: w1_chunk(jj) for jj in range(4)}

            xbf1_stack = contextlib.ExitStack()
            xbf1_pool = xbf1_stack.enter_context(
                tc.tile_pool(name="xbf1_pool", bufs=1, side="right"))
            xbf1 = xbf1_pool.tile([128, NQT, D], BF16, name="xbf1")

            def ln_stats(src_ap, qt):
                for half in range(2):
                    nc.vector.bn_stats(
                        out=stats[:, qt, half, :],
                        in_=src_ap[:, half * 512:half * 512 + 512])
                nc.vector.bn_aggr(out=mv2[:, qt, :], in_=stats[:, qt])
                nc.scalar.activation(out=sd[:, qt:qt + 1],
                                     in_=mv2[:, qt, 1:2], func=AF.Sqrt,
                                     bias=eps_t, scale=1.0)
                nc.vector.reciprocal(rstd[:, qt:qt + 1], sd[:, qt:qt + 1])

            with contextlib.ExitStack() as ph6:
                pm_pool = ph6.enter_context(tc.tile_pool(name="pm_pool",
                                                         bufs=2))
                ps_o = ph6.enter_context(
                    tc.tile_pool(name="ps_o", bufs=3, space="PSUM"))
                ps_sp = ph6.enter_context(
                    tc.tile_pool(name="ps_sp", bufs=3, space="PSUM"))
                ps_tr = ph6.enter_context(
                    tc.tile_pool(name="ps_tr", bufs=2, space="PSUM"))
                def xln_transpose(qt):
                    # transpose normalized qt block for ff1 (lagged one qt
                    # so the PE never waits on LN1's DVE chain)
                    qc = slice(qt * 128, qt * 128 + 128)
                    for fc in range(KC):
                        pst = ps_tr.tile([128, 128], BF16, name="pst",
                                         tag="pst")
                        nc.tensor.transpose(
                            pst, xbf1[:, qt, fc * 128:fc * 128 + 128],
                            ident_b)
                        nc.vector.tensor_copy(out=xlnT[:, fc, qc],
                                              in_=pst)

                with nc.named_scope("p6_fuse"):
                    for qt in range(NQT):
                        qc = slice(qt * 128, qt * 128 + 128)
                        # out_proj (2 halves, ctxT-stationary)
                        po0 = ps_o.tile([128, 512], F32, name="po", tag="po")
                        po1 = ps_o.tile([128, 512], F32, name="po", tag="po")
                        for kc in range(KC):
                            st, sp = (kc == 0), (kc == KC - 1)
                            nc.tensor.matmul(po0, ctxT[:, kc, qc],
                                             woT_s[:, kc, 0:512],
                                             start=st, stop=sp)
                            nc.tensor.matmul(po1, ctxT[:, kc, qc],
                                             woT_s[:, kc, 512:1024],
                                             start=st, stop=sp)
                        # masked-p transposes for this qt
                        pmt = pm_pool.tile([128, NTOK, 128], BF16, name="pmt",
                                           tag="pmt")
                        for t in range(NTOK):
                            pst = ps_tr.tile([128, 128], BF16, name="pst",
                                             tag="pst")
                            nc.tensor.transpose(
                                pst, psp[:, qt, t * 128:t * 128 + 128],
                                ident_b)
                            nc.vector.tensor_copy(out=pmt[:, t, :], in_=pst)
                        # spmm (2 halves)
                        sp0 = ps_sp.tile([128, 512], F32, name="psp2",
                                         tag="psp2")
                        sp1 = ps_sp.tile([128, 512], F32, name="psp2",
                                         tag="psp2")
                        for t in range(NTOK):
                            st, spl = (t == 0), (t == NTOK - 1)
                            nc.tensor.matmul(sp0, pmt[:, t, :],
                                             Vsp[:, t, 0:512],
                                             start=st, stop=spl)
                            nc.tensor.matmul(sp1, pmt[:, t, :],
                                             Vsp[:, t, 512:1024],
                                             start=st, stop=spl)
                        if qt >= 1:
                            xln_transpose(qt - 1)
                        # fuse on DVE: x1 = sig*dense + rcp*spmm + xot
                        xq = x1[:, qt, :]
                        nc.vector.tensor_scalar(
                            out=xq[:, 0:512], in0=po0, scalar1=sig_bc,
                            scalar2=None, op0=OP.mult)
                        nc.vector.tensor_scalar(
                            out=xq[:, 512:1024], in0=po1, scalar1=sig_bc,
                            scalar2=None, op0=OP.mult)
                        nc.vector.tensor_add(xq, xq, xot[:, qt, :])
                        nc.vector.scalar_tensor_tensor(
                            out=xq[:, 0:512], in0=sp0,
                            scalar=rcp_sp[:, qt:qt + 1],
                            in1=xq[:, 0:512], op0=OP.mult, op1=OP.add)
                        nc.vector.scalar_tensor_tensor(
                            out=xq[:, 512:1024], in0=sp1,
                            scalar=rcp_sp[:, qt:qt + 1],
                            in1=xq[:, 512:1024], op0=OP.mult, op1=OP.add)
                        # LN1 (keep x1 raw f32 for the ff2 residual)
                        ln_stats(xq, qt)
                        nc.vector.tensor_scalar(
                            out=xbf1[:, qt, :], in0=xq,
                            scalar1=mv2[:, qt, 0:1],
                            scalar2=rstd[:, qt:qt + 1],
                            op0=OP.subtract, op1=OP.mult)
                    xln_transpose(NQT - 1)

            xbf1_stack.close()
            wo_stack.close()
            xot_stack.close()
            av_stack.close()   # free Vaug, Vsp, ctxT
            sp_stack.close()   # free psp, kspT, qspT

            # f2T resident for qt-outer ff2 (chunk DMAs spread through ff1)
            f2_stack = contextlib.ExitStack()
            f2_pool = f2_stack.enter_context(
                tc.tile_pool(name="f2_pool", bufs=1))
            f2_s = f2_pool.tile([128, FC, D], BF16, name="f2_s")

            # xg = xhat*g1 + (be1+b2), computed on DVE during ff1
            xg = fse.tile([128, NQT, D], F32, name="xg")

            # ============ ff1 + relu ============
            h1_stack = contextlib.ExitStack()
            h1_pool = h1_stack.enter_context(
                tc.tile_pool(name="h1_pool", bufs=1))
            h1T = h1_pool.tile([128, FC, SQ], BF16, name="h1T")
            with contextlib.ExitStack() as ph9:
                ps_f1 = ph9.enter_context(
                    tc.tile_pool(name="ps_f1", bufs=4, space="PSUM"))
                with nc.named_scope("p9_ff1"):
                    for jj in range(16):
                        wt = w1_tiles.pop(jj)
                        if jj + 4 < 16:
                            w1_tiles[jj + 4] = w1_chunk(jj + 4)
                        for kc2 in range(2):
                            nc.gpsimd.dma_start(
                                out=f2_s[:, jj * 2 + kc2, :],
                                in_=f2T_r[:, jj * 2 + kc2, :])
                        for fi in range(2):
                            dft = jj * 2 + fi
                            ps = ps_f1.tile([128, 512], F32, name="ps_f",
                                            tag="ps_f")
                            for kc in range(KC):
                                nc.tensor.matmul(
                                    ps, wt[:, kc, fi * 128:fi * 128 + 128],
                                    xlnT[:, kc, :],
                                    start=(kc == 0), stop=(kc == KC - 1))
                            nc.scalar.activation(
                                out=h1T[:, dft, :], in_=ps, func=AF.Relu,
                                bias=b1_c[:, dft:dft + 1], scale=1.0)
                        if jj < 2 * NQT and jj % 2 == 1:
                            # xg for qt = jj//2, hidden under ff1
                            qt = jj // 2
                            nc.vector.tensor_scalar(
                                out=xg[:, qt, :], in0=x1[:, qt, :],
                                scalar1=mv2[:, qt, 0:1],
                                scalar2=rstd[:, qt:qt + 1],
                                op0=OP.subtract, op1=OP.mult)
                            nc.vector.tensor_mul(xg[:, qt, :], xg[:, qt, :],
                                                 g1_bc)
                            nc.vector.tensor_add(xg[:, qt, :], xg[:, qt, :],
                                                 b12_bc)
            w3_stack.close()
            xln_stack.close()

            # ============ ff2 (qt-outer) + residual + LN2 + out ============
            with contextlib.ExitStack() as ph10:
                ps_f2 = ph10.enter_context(
                    tc.tile_pool(name="ps_f2", bufs=4, space="PSUM"))
                ot_pool = ph10.enter_context(
                    tc.tile_pool(name="ot_pool", bufs=2))
                with nc.named_scope("p10_ff2"):
                    for qt in range(NQT):
                        qc = slice(qt * 128, qt * 128 + 128)
                        pg0 = ps_f2.tile([128, 512], F32, name="pg", tag="pg")
                        pg1 = ps_f2.tile([128, 512], F32, name="pg", tag="pg")
                        for kc in range(FC):
                            st, sp = (kc == 0), (kc == FC - 1)
                            nc.tensor.matmul(pg0, h1T[:, kc, qc],
                                             f2_s[:, kc, 0:512],
                                             start=st, stop=sp)
                            nc.tensor.matmul(pg1, h1T[:, kc, qc],
                                             f2_s[:, kc, 512:1024],
                                             start=st, stop=sp)
                        x2 = x1[:, qt, :]
                        nc.vector.tensor_add(x2[:, 0:512], pg0,
                                             xg[:, qt, 0:512])
                        nc.vector.tensor_add(x2[:, 512:1024], pg1,
                                             xg[:, qt, 512:1024])
                        ln_stats(x2, qt)
                        ot = ot_pool.tile([128, D], F32, name="out_t",
                                          tag="out_t")
                        nc.vector.tensor_scalar(
                            out=ot, in0=x2, scalar1=mv2[:, qt, 0:1],
                            scalar2=rstd[:, qt:qt + 1],
                            op0=OP.subtract, op1=OP.mult)
                        nc.vector.tensor_mul(ot, ot, g2_bc)
                        nc.vector.tensor_add(ot, ot, be2_bc)
                        nc.scalar.dma_start(
                            out=out.ap()[qt * 128:qt * 128 + 128, :], in_=ot)
            h1_stack.close()
            f2_stack.close()

    nc.compile()
    return nc


def _prep_inputs(src, in_proj_w, in_proj_b, out_proj_w, out_proj_b,
                 Qp_w, Qp_b, Kp_w, Kp_b, Vp_w, Vp_b, lam,
                 ff1_w, ff1_b, ff2_w, ff2_b, ln1_g, ln1_b, ln2_g, ln2_b):
    import ml_dtypes
    f = np.float32
    A = lambda x: np.ascontiguousarray(x, dtype=f)
    AB = lambda x: np.ascontiguousarray(np.asarray(x, dtype=f),
                                        dtype=ml_dtypes.bfloat16)
    in_proj_w = np.asarray(in_proj_w, dtype=f)
    ff1_w = np.asarray(ff1_w, dtype=f)
    ln1_g = np.asarray(ln1_g, dtype=f)
    ln1_b = np.asarray(ln1_b, dtype=f)
    # fold ln1 gamma into ff1 weights, ln1 beta into ff1 bias
    f1_eff = ff1_w * ln1_g[None, :]            # [DFF, D]
    b1_eff = np.asarray(ff1_b, dtype=f) + ff1_w @ ln1_b

    # packed per-partition bias columns [128, 50]
    bcols = np.zeros((128, 50), dtype=f)
    qk_bias = np.asarray(in_proj_b, dtype=f)
    bcols[:, 0:8] = qk_bias[0:D].reshape(8, 128).T
    bcols[:, 8:16] = qk_bias[D:2 * D].reshape(8, 128).T
    bcols[:, 16:48] = b1_eff.reshape(32, 128).T
    bcols[0:64, 48] = np.asarray(Qp_b, dtype=f)
    bcols[0:64, 49] = np.asarray(Kp_b, dtype=f)

    # packed broadcast rows [1, 7D]: bv, bvp, bo, b12, g1, g2, be2
    brow = np.concatenate([
        qk_bias[2 * D:3 * D],
        np.asarray(Vp_b, dtype=f),
        np.asarray(out_proj_b, dtype=f),
        ln1_b + np.asarray(ff2_b, dtype=f),
        ln1_g,
        np.asarray(ln2_g, dtype=f),
        np.asarray(ln2_b, dtype=f),
    ]).reshape(1, 7 * D)

    # qkp packed partition-major: [128, kc, 2R] contiguous
    qkpT = np.concatenate([np.asarray(Qp_w).T, np.asarray(Kp_w).T], axis=1)
    qkp_pm = np.ascontiguousarray(
        np.asarray(qkpT, dtype=f).reshape(KC, 128, 2 * R)
        .transpose(1, 0, 2).reshape(128, -1),
        dtype=ml_dtypes.bfloat16)

    shared = {
        "wqkvT": AB(in_proj_w.T),
        "woT": AB(np.asarray(out_proj_w).T),
        "vpT": AB(np.asarray(Vp_w).T),
        "qkp_pm": qkp_pm,
        "f1T": AB(f1_eff.T),
        "f2T": AB(np.asarray(ff2_w).T),
        "bcols": bcols,
        "brow": A(brow),
        "lam": A(np.asarray(lam)).reshape(1, 1),
    }
    in_maps = []
    for core in range(8):
        b, h = core // 2, core % 2
        srcb = np.asarray(src[b])
        xTb = srcb.T
        if h == 1:
            # own-query columns first (key order is irrelevant to attention)
            xTb = np.concatenate([xTb[:, SQ:], xTb[:, :SQ]], axis=1)
        m = dict(shared)
        m["xT"] = AB(xTb)
        m["x_own"] = A(srcb[h * SQ:(h + 1) * SQ, :])
        in_maps.append(m)
    return in_maps


def _run(inputs, trace=False):
    if "nc" not in _cached:
        _cached["nc"] = _build()
    nc = _cached["nc"]
    in_maps = _prep_inputs(**inputs)
    res = run_bass_kernel_spmd(nc, in_maps, core_ids=list(range(8)),
                               trace=trace)
    out = np.empty((B, S, D), np.float32)
    for core in range(8):
        b, h = core // 2, core % 2
        out[b, h * SQ:(h + 1) * SQ, :] = res.results[core]["out"]
    return out, res


def kernel(**inputs) -> np.ndarray:
    out, _ = _run(inputs, trace=False)
    return out
